# revision 1
# baseline (speedup 1.0000x reference)
"""Trainium2 Bass kernel for nn_GNN_GRU_83519934038653 (GatedGraphConv-style GNN).

Strategy (8 NeuronCores, SPMD, one NEFF):
  - Host: sort edges by dst, shard edges by dst node-range (1280 nodes/core so the
    per-core GRU output shards concatenate exactly like AllGather output), build
    int16 gather indices + per-tile one-hot scatter matrices S, permute weights.
  - Phase A (device): h0 = x@W_emb+b (replicated); per-edge he / g (edge MLP);
    We = g@W2+b2 materialized in DRAM as fp16 in (o,i)-permuted layout.
  - Phase B x3 (device): dma_gather h[src] -> DVE multiply (h broadcast over o) ->
    one tree-add level -> PE matmul with one-hot S as stationary operand, which
    simultaneously finishes the i-reduction (16 partial lanes ride the moving
    free dim) and performs the dst segment-sum into PSUM windows -> per-window
    tree -> transposed GRU (PE matmuls, ACT sigmoid/tanh, DVE elementwise) ->
    AllGather the fp16 h shard across the 8 cores.
"""

import os
import sys

for _p in ("/opt/trn_rl_repo", "/root/.axon_site/_ro/trn_rl_repo"):
    if os.path.isdir(_p) and _p not in sys.path:
        sys.path.insert(0, _p)

import numpy as np

import concourse.bass as bass
import concourse.bacc as bacc
import concourse.mybir as mybir
import concourse.tile as tile
import concourse.bass_utils as bass_utils
from concourse.masks import make_identity

F32 = mybir.dt.float32
F16 = mybir.dt.float16
F8 = mybir.dt.float8e4
I16 = mybir.dt.int16
AF = mybir.ActivationFunctionType
OP = mybir.AluOpType

# problem constants (hardcoded per contest rules)
N_NODES = 10000
N_EDGES = 160000
D = 32
IN_DIM = 2
STEPS = 3
CORES = 8
NPC = 1280  # padded nodes per core; 8*1280 = 10240 >= 10000
WIN = 128   # scatter window (nodes per PSUM accumulation window)

S_DT = F8           # dtype of the one-hot scatter matrices (exact for 0/1)
S_NP = mybir.dt.np(F8)

LAST_RESULT = None  # BassKernelResults of the most recent run (for test.py)


# --------------------------------------------------------------------------- #
# host-side preparation
# --------------------------------------------------------------------------- #

def _wrap_idx(idx, epc):
    """dma_gather index layout: idx j lives at [j % 16, j // 16], replicated to
    128 partitions."""
    w = idx.reshape(epc // 16, 16).T.astype(np.int16)  # [16, epc/16]
    return np.tile(w, (8, 1)).copy()                   # [128, epc/16]


def host_prep(x, src, dst, W_emb, b_emb, W_theta, b_theta, W_phi, b_phi,
              W1, b1, W2, b2, W_ih, b_ih, W_hh, b_hh,
              n_nodes=N_NODES, npc=NPC, ncores=CORES):
    """Returns (shared_inputs, per_core_inputs, meta)."""
    n_pad = npc * ncores
    wpc = npc // WIN           # windows per core
    nwin = ncores * wpc

    src = np.asarray(src).astype(np.int64)
    dst = np.asarray(dst).astype(np.int64)
    order = np.argsort(dst, kind="stable")
    src_s = src[order]
    dst_s = dst[order]

    win_of_edge = dst_s // WIN
    counts = np.bincount(win_of_edge, minlength=nwin)
    T = max(1, int(np.ceil(counts.max() / 128)))
    ntpc = wpc * T             # tiles per core
    epc = ntpc * 128           # padded edges per core

    win_start = np.concatenate([[0], np.cumsum(counts)])

    per_core = []
    for c in range(ncores):
        src_idx = np.zeros(epc, np.int64)
        dst_idx = np.zeros(epc, np.int64)
        S = np.zeros((128, ntpc, 128), np.float32)
        for w in range(wpc):
            g = c * wpc + w
            e0, e1 = int(win_start[g]), int(win_start[g + 1])
            k = e1 - e0
            if k == 0:
                continue
            base = w * T * 128
            j = np.arange(k)
            src_idx[base + j] = src_s[e0:e1]
            dst_idx[base + j] = dst_s[e0:e1]
            dloc = dst_s[e0:e1] - (c * npc + w * WIN)
            assert (dloc >= 0).all() and (dloc < WIN).all()
            S[j % 128, w * T + j // 128, dloc] = 1.0
        per_core.append({
            "src_w": _wrap_idx(src_idx, epc),
            "dst_w": _wrap_idx(dst_idx, epc),
            "s_mat": S.astype(S_NP),
            "xT_own": None,  # filled below
        })

    # weight transforms
    f32 = np.float32
    f16 = np.float16
    x = np.asarray(x, f32)
    x_pad = np.zeros((n_pad, IN_DIM), f32)
    x_pad[:n_nodes] = x
    xT_aug = np.concatenate([x_pad.T, np.ones((1, n_pad), f32)], 0)  # [3, n_pad]
    for c in range(ncores):
        per_core[c]["xT_own"] = np.ascontiguousarray(
            xT_aug[:, c * npc:(c + 1) * npc])

    W_emb_aug = np.concatenate([np.asarray(W_emb, f32),
                                np.asarray(b_emb, f32)[None, :]], 0)  # [3, 32]

    W2p = np.asarray(W2, f32).reshape(64, D, D).transpose(0, 2, 1).reshape(64, D * D)
    b2p = np.asarray(b2, f32).reshape(D, D).T.reshape(D * D)
    W2pa = np.concatenate([W2p, b2p[None, :]], 0).astype(f16)  # [65, 1024]

    shared = {
        "xT_aug": xT_aug,
        "w_emb": W_emb_aug,
        "w_theta": np.asarray(W_theta, f32).astype(f16),
        "w_phi": np.asarray(W_phi, f32).astype(f16),
        "w1": np.asarray(W1, f32).astype(f16),
        "w2pa": W2pa,
        "w_ih": np.asarray(W_ih, f32),
        "w_hh": np.asarray(W_hh, f32),
        "b_tp": (np.asarray(b_theta, f32) + np.asarray(b_phi, f32))[:, None],
        "b1c": np.asarray(b1, f32)[:, None],
        "b_r": (np.asarray(b_ih, f32)[0:D] + np.asarray(b_hh, f32)[0:D])[:, None],
        "b_z": (np.asarray(b_ih, f32)[D:2 * D] + np.asarray(b_hh, f32)[D:2 * D])[:, None],
        "b_in": np.asarray(b_ih, f32)[2 * D:3 * D][:, None],
        "b_hn": np.asarray(b_hh, f32)[2 * D:3 * D][:, None],
    }
    meta = dict(T=T, npc=npc, ncores=ncores, n_pad=n_pad, wpc=wpc,
                ntpc=ntpc, epc=epc, steps=STEPS)
    return shared, per_core, meta


# --------------------------------------------------------------------------- #
# device kernel builder
# --------------------------------------------------------------------------- #

def _bcast_mid(ap_base, count):
    """Insert a [0, count] broadcast dim before the innermost dim of an AP."""
    aps = [list(p) for p in ap_base.ap]
    new = aps[:-1] + [[0, count]] + [aps[-1]]
    return bass.AP(ap_base.tensor, ap_base.offset, new)


def build_nc(meta):
    T = meta["T"]; npc = meta["npc"]; ncores = meta["ncores"]
    n_pad = meta["n_pad"]; wpc = meta["wpc"]; ntpc = meta["ntpc"]
    epc = meta["epc"]; steps = meta["steps"]
    steps_exec = int(os.environ.get("K_STEPS", steps))
    skip_a2 = bool(int(os.environ.get("K_NO_A2", "0")))

    nc = bacc.Bacc("TRN2", target_bir_lowering=False, debug=False,
                   enable_asserts=False, num_devices=ncores)

    # ---- I/O tensors ----
    t_xT = nc.dram_tensor("xT_aug", [IN_DIM + 1, n_pad], F32, kind="ExternalInput")
    t_xTo = nc.dram_tensor("xT_own", [IN_DIM + 1, npc], F32, kind="ExternalInput")
    t_wemb = nc.dram_tensor("w_emb", [IN_DIM + 1, D], F32, kind="ExternalInput")
    t_wth = nc.dram_tensor("w_theta", [D, D], F16, kind="ExternalInput")
    t_wph = nc.dram_tensor("w_phi", [D, D], F16, kind="ExternalInput")
    t_w1 = nc.dram_tensor("w1", [D, 64], F16, kind="ExternalInput")
    t_w2 = nc.dram_tensor("w2pa", [65, 1024], F16, kind="ExternalInput")
    t_wih = nc.dram_tensor("w_ih", [D, 3 * D], F32, kind="ExternalInput")
    t_whh = nc.dram_tensor("w_hh", [D, 3 * D], F32, kind="ExternalInput")
    t_btp = nc.dram_tensor("b_tp", [D, 1], F32, kind="ExternalInput")
    t_b1c = nc.dram_tensor("b1c", [64, 1], F32, kind="ExternalInput")
    t_br = nc.dram_tensor("b_r", [D, 1], F32, kind="ExternalInput")
    t_bz = nc.dram_tensor("b_z", [D, 1], F32, kind="ExternalInput")
    t_bin = nc.dram_tensor("b_in", [D, 1], F32, kind="ExternalInput")
    t_bhn = nc.dram_tensor("b_hn", [D, 1], F32, kind="ExternalInput")
    t_srcw = nc.dram_tensor("src_w", [128, epc // 16], I16, kind="ExternalInput")
    t_dstw = nc.dram_tensor("dst_w", [128, epc // 16], I16, kind="ExternalInput")
    t_smat = nc.dram_tensor("s_mat", [128, ntpc, 128], S_DT, kind="ExternalInput")
    t_out = nc.dram_tensor("out_h", [npc, D], F32, kind="ExternalOutput")

    with tile.TileContext(nc) as tc:
        with tc.tile_pool(name="dram", bufs=1, space="DRAM") as dpool, \
             tc.tile_pool(name="const", bufs=1) as cpool:
            we_dram = dpool.tile([epc, 1024], F16, name="we_dram")
            # one h tensor per step: Shared DRAM allows only a single writer
            # instruction, so the h0 DMA and each step's AllGather get their own.
            h_fulls = [dpool.tile([n_pad, 128], F16,
                                  addr_space=("Local" if s == 0 else "Shared"),
                                  name=f"h_full{s}") for s in range(steps)]
            cc_ins = [dpool.tile([npc, 128], F16, name=f"cc_in{s}")
                      for s in range(steps - 1)]

            # resident constants
            idm = cpool.tile([128, 128], F32, name="idm")
            make_identity(nc, idm[:])
            S_sb = cpool.tile([128, ntpc * 128], S_DT, name="S_sb")
            nc.sync.dma_start(S_sb[:], t_smat.ap().rearrange("p t e -> p (t e)"))
            isrc = cpool.tile([128, epc // 16], I16, name="isrc")
            nc.sync.dma_start(isrc[:], t_srcw.ap())
            idst = cpool.tile([128, epc // 16], I16, name="idst")
            nc.sync.dma_start(idst[:], t_dstw.ap())

            def load_const(t, shape, dtype, name):
                s = cpool.tile(shape, dtype, name=name)
                nc.sync.dma_start(s[:], t.ap())
                return s

            xTo_sb = load_const(t_xTo, [IN_DIM + 1, npc], F32, "xTo_sb")
            wemb_sb = load_const(t_wemb, [IN_DIM + 1, D], F32, "wemb_sb")
            wth_sb = load_const(t_wth, [D, D], F16, "wth_sb")
            wph_sb = load_const(t_wph, [D, D], F16, "wph_sb")
            w1_sb = load_const(t_w1, [D, 64], F16, "w1_sb")
            w2_sb = load_const(t_w2, [65, 1024], F16, "w2_sb")
            wih_sb = load_const(t_wih, [D, 3 * D], F32, "wih_sb")
            whh_sb = load_const(t_whh, [D, 3 * D], F32, "whh_sb")
            btp_sb = load_const(t_btp, [D, 1], F32, "btp_sb")
            b1c_sb = load_const(t_b1c, [64, 1], F32, "b1c_sb")
            br_sb = load_const(t_br, [D, 1], F32, "br_sb")
            bz_sb = load_const(t_bz, [D, 1], F32, "bz_sb")
            bin_sb = load_const(t_bin, [D, 1], F32, "bin_sb")
            bhn_sb = load_const(t_bhn, [D, 1], F32, "bhn_sb")

            # GRU state (transposed layout), ping-pong across steps
            h_bufs = [cpool.tile([D, npc], F32, name=f"hT{i}") for i in range(2)]

            # ---------------- Phase A1: h0 ----------------
            with tc.tile_pool(name="pA1", bufs=2, space="PSUM") as pp1, \
                 tc.tile_pool(name="sA1", bufs=3) as sp1:
                nch = n_pad // 128
                h0st = cpool.tile([128, nch, D], F16, name="h0st")
                for ch in range(nch):
                    xc = sp1.tile([IN_DIM + 1, 128], F32, tag="xc")
                    nc.sync.dma_start(xc[:], t_xT.ap()[:, ch * 128:(ch + 1) * 128])
                    ps = pp1.tile([128, D], F32, tag="psh0")
                    nc.tensor.matmul(ps[:], lhsT=xc[:],
                                     rhs=wemb_sb[:], start=True, stop=True)
                    nc.scalar.copy(h0st[:, ch, :], ps[:])
                DCH = 16  # tiles per DMA (<=2048 row descriptors each)
                for d0 in range(0, nch, DCH):
                    dn = min(DCH, nch - d0)
                    nc.sync.dma_start(
                        h_fulls[0][d0 * 128:(d0 + dn) * 128, 0:D]
                        .rearrange("(t p) d -> p t d", p=128),
                        h0st[:, d0:d0 + dn, :])
                # own-range h0, transposed (f32 GRU state)
                for c0 in range(0, npc, 512):
                    cn = min(512, npc - c0)
                    ps = pp1.tile([D, 512], F32, tag="pshT")
                    nc.tensor.matmul(ps[:, 0:cn], lhsT=wemb_sb[:],
                                     rhs=xTo_sb[:, c0:c0 + cn], start=True, stop=True)
                    nc.vector.tensor_copy(h_bufs[0][:, c0:c0 + cn], ps[:, 0:cn])

            # ---------------- Phase A2: edge MLP -> We ----------------
            CT = int(os.environ.get("K_CT", "32")) if not skip_a2 else 0  # tiles per gather chunk; 0 skips A2
            with tc.tile_pool(name="pA2", bufs=2, space="PSUM") as pp2, \
                 tc.tile_pool(name="pW", bufs=2, space="PSUM") as pw, \
                 tc.tile_pool(name="sA2", bufs=2) as sp2, \
                 tc.tile_pool(name="sWt", bufs=3) as spw:
                evac_flip = 0
                for t0 in range(0, ntpc, CT) if CT else []:
                    tn = min(CT, ntpc - t0)
                    e0, en = t0 * 128, tn * 128
                    gs = sp2.tile([128, CT * 128], F16, tag="gsrcT")
                    gd = sp2.tile([128, CT * 128], F16, tag="gdstT")
                    nc.gpsimd.dma_gather(
                        gs[:, 0:en].rearrange("p (o e) -> p o e", o=1),
                        h_fulls[0][:, :], isrc[:, e0 // 16:(e0 + en) // 16],
                        en, en, 128, transpose=True, single_packet=False)
                    nc.gpsimd.dma_gather(
                        gd[:, 0:en].rearrange("p (o e) -> p o e", o=1),
                        h_fulls[0][:, :], idst[:, e0 // 16:(e0 + en) // 16],
                        en, en, 128, transpose=True, single_packet=False)
                    dT = sp2.tile([D, CT * 128], F16, tag="dT")
                    nc.vector.tensor_sub(dT[:, 0:en], gd[0:D, 0:en], gs[0:D, 0:en])
                    for j0 in range(0, en, 512):
                        n = min(512, en - j0)
                        psh = pp2.tile([D, 512], F32, tag="pshe")
                        nc.tensor.matmul(psh[:, 0:n], lhsT=wth_sb[:],
                                         rhs=dT[:, j0:j0 + n], start=True, stop=False)
                        nc.tensor.matmul(psh[:, 0:n], lhsT=wph_sb[:],
                                         rhs=gs[0:D, j0:j0 + n], start=False, stop=True)
                        he = sp2.tile([D, 512], F16, tag="he")
                        nc.scalar.activation(he[:, 0:n], psh[:, 0:n], AF.Relu,
                                             bias=btp_sb[:])
                        psg = pp2.tile([64, 512], F32, tag="psg")
                        nc.tensor.matmul(psg[:, 0:n], lhsT=w1_sb[:], rhs=he[:, 0:n],
                                         start=True, stop=True)
                        ga = sp2.tile([65, 512], F16, tag="ga")
                        nc.vector.memset(ga[64:65, 0:n], 1.0)
                        nc.scalar.activation(ga[0:64, 0:n], psg[:, 0:n], AF.Relu,
                                             bias=b1c_sb[:])
                        for s0 in range(0, n, 128):
                            gt = t0 + (j0 + s0) // 128
                            pw0 = pw.tile([128, 512], F32, tag="psw0")
                            pw1 = pw.tile([128, 512], F32, tag="psw1")
                            nc.tensor.matmul(pw0[:], lhsT=ga[:, s0:s0 + 128],
                                             rhs=w2_sb[:, 0:512], start=True, stop=True)
                            nc.tensor.matmul(pw1[:], lhsT=ga[:, s0:s0 + 128],
                                             rhs=w2_sb[:, 512:1024], start=True, stop=True)
                            wt = spw.tile([128, 1024], F16, tag="wt")
                            if evac_flip == 0:
                                nc.scalar.copy(wt[:, 0:512], pw0[:])
                                nc.vector.tensor_copy(wt[:, 512:1024], pw1[:])
                            else:
                                nc.vector.tensor_copy(wt[:, 0:512], pw0[:])
                                nc.scalar.copy(wt[:, 512:1024], pw1[:])
                            evac_flip ^= 1
                            nc.sync.dma_start(
                                we_dram[gt * 128:(gt + 1) * 128, :], wt[:])

            # ---------------- Phase B: message passing steps ----------------
            we_view = we_dram[:].rearrange("(t p) f -> p t f", p=128)
            with tc.tile_pool(name="sG", bufs=1) as sg, \
                 tc.tile_pool(name="sWq", bufs=2) as swq, \
                 tc.tile_pool(name="sPr", bufs=2) as spr, \
                 tc.tile_pool(name="sP16", bufs=2) as sp16, \
                 tc.tile_pool(name="sWin", bufs=2) as swin, \
                 tc.tile_pool(name="sGru", bufs=1) as sgru, \
                 tc.tile_pool(name="pA", bufs=2, space="PSUM") as ppa, \
                 tc.tile_pool(name="pT", bufs=1, space="PSUM") as ppt, \
                 tc.tile_pool(name="pG", bufs=1, space="PSUM") as ppg:
                for step in range(steps_exec) if steps_exec else []:
                    h_cur = h_bufs[step % 2]
                    h_new = h_bufs[(step + 1) % 2]

                    G = sg.tile([128, ntpc, 128], F16, tag="G")
                    nsplit = 3 if ntpc >= 3 else 1
                    bnds = [round(i * ntpc / nsplit) for i in range(nsplit + 1)]
                    for ta, tb in zip(bnds[:-1], bnds[1:]):
                        if tb > ta:
                            nc.gpsimd.dma_gather(
                                G[:, ta:tb, :], h_fulls[step][:, :],
                                isrc[:, ta * 8:tb * 8],
                                (tb - ta) * 128, (tb - ta) * 128, 128,
                                transpose=False, single_packet=False)

                    aT = sgru.tile([D, npc], F32, tag="aT")
                    psa = None
                    for q0 in range(0, ntpc, 4):
                        k = min(4, ntpc - q0)
                        wq = swq.tile([128, 4, 1024], F16, tag="wq")
                        nc.sync.dma_start(wq[:, 0:k, :], we_view[:, q0:q0 + k, :])
                        prod = spr.tile([128, 4, D, D], F16, tag="prod")
                        base = G[:, q0:q0 + k, 0:D]
                        in1 = _bcast_mid(base, D)
                        nc.vector.tensor_tensor(
                            prod[:, 0:k, :, :],
                            wq[:, 0:k, :].rearrange("p t (o i) -> p t o i", o=D),
                            in1, op=OP.mult)
                        p16 = sp16.tile([128, 4, D, 16], F16, tag="p16")
                        nc.vector.tensor_tensor(
                            p16[:, 0:k, :, :], prod[:, 0:k, :, 0:16],
                            prod[:, 0:k, :, 16:32], op=OP.add)
                        for j in range(k):
                            gt = q0 + j
                            w = gt // T
                            tloc = gt % T
                            if tloc == 0:
                                psa = ppa.tile([128, 512], F32, tag="psa")
                            nc.tensor.matmul(
                                psa[:], lhsT=S_sb[:, gt * 128:(gt + 1) * 128],
                                rhs=p16[:, j, :, :],
                                start=(tloc == 0), stop=(tloc == T - 1))
                            if tloc == T - 1:
                                # evacuate window w: finish i-reduction + transpose
                                aw = swin.tile([128, D, 16], F32, tag="aw")
                                nc.scalar.copy(
                                    aw[:], psa[:].rearrange("p (o i) -> p o i", o=D))
                                t8 = swin.tile([128, D, 8], F32, tag="t8")
                                nc.vector.tensor_tensor(t8[:], aw[:, :, 0:8],
                                                        aw[:, :, 8:16], op=OP.add)
                                t4 = swin.tile([128, D, 4], F32, tag="t4")
                                nc.vector.tensor_tensor(t4[:], t8[:, :, 0:4],
                                                        t8[:, :, 4:8], op=OP.add)
                                t2 = swin.tile([128, D, 2], F32, tag="t2")
                                nc.vector.tensor_tensor(t2[:], t4[:, :, 0:2],
                                                        t4[:, :, 2:4], op=OP.add)
                                t1 = swin.tile([128, D], F32, tag="t1")
                                nc.vector.tensor_tensor(t1[:], t2[:, :, 0],
                                                        t2[:, :, 1], op=OP.add)
                                pst = ppt.tile([D, 128], F32, tag="pst")
                                nc.tensor.transpose(pst[:], t1[:], idm[:])
                                nc.vector.tensor_copy(
                                    aT[:, w * 128:(w + 1) * 128], pst[:])

                    # ---- GRU (transposed layout) ----
                    for c0 in range(0, npc, 512):
                        cn = min(512, npc - c0)
                        cs = slice(c0, c0 + cn)
                        pgi = ppg.tile([3 * D, 512], F32, tag="pgi")
                        nc.tensor.matmul(pgi[:, 0:cn], lhsT=wih_sb[:],
                                         rhs=aT[:, cs], start=True, stop=True)
                        pgh = ppg.tile([3 * D, 512], F32, tag="pgh")
                        nc.tensor.matmul(pgh[:, 0:cn], lhsT=whh_sb[:],
                                         rhs=h_cur[:, cs], start=True, stop=True)
                        gh_sb = sgru.tile([3 * D, 512], F32, tag="gh_sb")
                        nc.scalar.copy(gh_sb[:, 0:cn], pgh[:, 0:cn])
                        tr = sgru.tile([D, 512], F32, tag="tr")
                        nc.vector.tensor_add(tr[:, 0:cn], pgi[0:D, 0:cn],
                                             gh_sb[0:D, 0:cn])
                        r = sgru.tile([D, 512], F32, tag="r")
                        nc.scalar.activation(r[:, 0:cn], tr[:, 0:cn], AF.Sigmoid,
                                             bias=br_sb[:])
                        tz = sgru.tile([D, 512], F32, tag="tz")
                        nc.vector.tensor_add(tz[:, 0:cn], pgi[D:2 * D, 0:cn],
                                             gh_sb[D:2 * D, 0:cn])
                        z = sgru.tile([D, 512], F32, tag="z")
                        nc.scalar.activation(z[:, 0:cn], tz[:, 0:cn], AF.Sigmoid,
                                             bias=bz_sb[:])
                        hnb = sgru.tile([D, 512], F32, tag="hnb")
                        nc.vector.tensor_scalar_add(hnb[:, 0:cn],
                                                    gh_sb[2 * D:3 * D, 0:cn],
                                                    bhn_sb[:])
                        rhn = sgru.tile([D, 512], F32, tag="rhn")
                        nc.vector.tensor_mul(rhn[:, 0:cn], r[:, 0:cn], hnb[:, 0:cn])
                        tn_ = sgru.tile([D, 512], F32, tag="tn_")
                        nc.vector.tensor_add(tn_[:, 0:cn], rhn[:, 0:cn],
                                             pgi[2 * D:3 * D, 0:cn])
                        ngate = sgru.tile([D, 512], F32, tag="ngate")
                        nc.scalar.activation(ngate[:, 0:cn], tn_[:, 0:cn], AF.Tanh,
                                             bias=bin_sb[:])
                        hmn = sgru.tile([D, 512], F32, tag="hmn")
                        nc.vector.tensor_sub(hmn[:, 0:cn], h_cur[:, cs],
                                             ngate[:, 0:cn])
                        zh = sgru.tile([D, 512], F32, tag="zh")
                        nc.vector.tensor_mul(zh[:, 0:cn], z[:, 0:cn], hmn[:, 0:cn])
                        nc.vector.tensor_add(h_new[:, cs], ngate[:, 0:cn],
                                             zh[:, 0:cn])

                    # ---- write h out; AllGather (except after last step) ----
                    if step < steps_exec - 1:
                        hst = sgru.tile([128, wpc, D], F16, tag="hst")
                        for w in range(wpc):
                            ps2 = ppt.tile([128, D], F32, tag="ps2")
                            nc.tensor.transpose(ps2[:],
                                                h_new[:, w * 128:(w + 1) * 128],
                                                idm[0:D, 0:D])
                            nc.scalar.copy(hst[:, w, :], ps2[:])
                        nc.sync.dma_start(
                            cc_ins[step][:, 0:D].rearrange("(w p) d -> p w d", p=128),
                            hst[:])
                        nc.gpsimd.collective_compute(
                            "AllGather", OP.bypass,
                            replica_groups=[list(range(ncores))],
                            ins=[cc_ins[step][:].opt()], outs=[h_fulls[step + 1][:].opt()])
                    else:
                        ost = sgru.tile([128, wpc, D], F32, tag="ost")
                        for w in range(wpc):
                            ps2 = ppt.tile([128, D], F32, tag="ps2")
                            nc.tensor.transpose(ps2[:],
                                                h_new[:, w * 128:(w + 1) * 128],
                                                idm[0:D, 0:D])
                            nc.scalar.copy(ost[:, w, :], ps2[:])
                        nc.sync.dma_start(
                            t_out.ap().rearrange("(w p) d -> p w d", p=128), ost[:])

        if steps_exec == 0:
            with tc.tile_pool(name="sZ", bufs=1) as sz:
                zst = sz.tile([128, wpc, D], F32, name="zst")
                nc.vector.memset(zst[:], 0.0)
                nc.sync.dma_start(
                    t_out.ap().rearrange("(w p) d -> p w d", p=128), zst[:])

    nc.compile()
    return nc


# --------------------------------------------------------------------------- #
# entry point
# --------------------------------------------------------------------------- #

LAST_META = None


def run(inputs, n_nodes=N_NODES, npc=NPC, **spmd_kwargs):
    global LAST_RESULT, LAST_META
    shared, per_core, meta = host_prep(**inputs, n_nodes=n_nodes, npc=npc)
    LAST_META = meta
    nc = build_nc(meta)
    in_maps = [dict(shared, **pc) for pc in per_core]
    res = bass_utils.run_bass_kernel_spmd(
        nc, in_maps, core_ids=list(range(meta["ncores"])), **spmd_kwargs)
    LAST_RESULT = res
    out = np.concatenate([res.results[c]["out_h"] for c in range(meta["ncores"])], 0)
    return np.ascontiguousarray(out[:n_nodes]).astype(np.float32)


def kernel(**inputs):
    return run(inputs)



# revision 22
# speedup vs baseline: 1.2490x; 1.2490x over previous
"""Trainium2 Bass kernel for nn_GNN_GRU_83519934038653 (GatedGraphConv-style GNN).

Strategy (8 NeuronCores, SPMD, one NEFF):
  - Host: sort edges by dst, shard by dst node-range (1280 nodes/core), build
    int16 gather index tables + per-tile one-hot scatter matrices S (and their
    per-tile transposes ST for the dst-side select), permute weights.
  - h tables in DRAM use a host-chosen row permutation so every SBUF->DRAM h
    write is one contiguous descriptor per partition; the gather index tables
    absorb the permutation.
  - Gathers run as prepare_only descriptor generation on SWDGE queues 1-3
    (hidden under compute) + trigger_dma when the source table is ready;
    pre-staged descriptors drain at ~250 GB/s vs ~40 GB/s gen-paced.
  - A2: one gather of h0[src] shared with step 1; h0[dst] needs no gather at
    all (dst is window-local: one-hot ST matmuls against h0 windows).
  - Phase B x3: wq = We chunk DMA; DVE multiply (h broadcast over o);
    one tree-add level; PE scatter-matmul with one-hot S; per-window tree;
    transposed GRU; AllGather h (except last step).
"""

import os
import sys

for _p in ("/opt/trn_rl_repo", "/root/.axon_site/_ro/trn_rl_repo"):
    if os.path.isdir(_p) and _p not in sys.path:
        sys.path.insert(0, _p)

import numpy as np

import concourse.bass as bass
import concourse.bacc as bacc
import concourse.mybir as mybir
import concourse.tile as tile
import concourse.bass_utils as bass_utils

F32 = mybir.dt.float32
F16 = mybir.dt.float16
F8 = mybir.dt.float8e4
I16 = mybir.dt.int16
AF = mybir.ActivationFunctionType
OP = mybir.AluOpType

N_NODES = 10000
N_EDGES = 160000
D = 32
IN_DIM = 2
STEPS = 3
CORES = 8
NPC = 1280   # padded nodes per core; 8*1280 = 10240
WIN = 128    # scatter window (nodes per PSUM accumulation window)
NCH = 3      # gather chunks (SWDGE queues 1..NCH)

S_NP = mybir.dt.np(F8)

LAST_RESULT = None
LAST_META = None


# --------------------------------------------------------------------------- #
# host-side preparation
# --------------------------------------------------------------------------- #

def _wrap_idx(idx, epc):
    w = idx.reshape(epc // 16, 16).T.astype(np.int16)
    return np.tile(w, (8, 1)).copy()


def _rowmap_A(n):
    """h_full0 row of node n: h0st[p, w] holds node w*128+p, stored p-major."""
    return (n % 128) * 80 + n // 128


def _rowmap_B(n):
    """h_full(step) row of node n after AllGather of p-major cc_in shards."""
    c = n // NPC
    loc = n % NPC
    return c * NPC + (loc % 128) * (NPC // 128) + loc // 128


def host_prep(x, src, dst, W_emb, b_emb, W_theta, b_theta, W_phi, b_phi,
              W1, b1, W2, b2, W_ih, b_ih, W_hh, b_hh,
              n_nodes=N_NODES, npc=NPC, ncores=CORES):
    n_pad = npc * ncores
    wpc = npc // WIN
    nwin = ncores * wpc

    src = np.asarray(src).astype(np.int64)
    dst = np.asarray(dst).astype(np.int64)
    order = np.argsort(dst, kind="stable")
    src_s = src[order]
    dst_s = dst[order]

    win_of_edge = dst_s // WIN
    counts = np.bincount(win_of_edge, minlength=nwin)
    T = max(1, int(np.ceil(counts.max() / 128)))
    ntpc = wpc * T
    epc = ntpc * 128
    assert epc % (16 * NCH) == 0 and ntpc % NCH == 0

    win_start = np.concatenate([[0], np.cumsum(counts)])

    per_core = []
    for c in range(ncores):
        src_idx = np.zeros(epc, np.int64)
        S = np.zeros((128, ntpc, 128), np.float32)
        for w in range(wpc):
            g = c * wpc + w
            e0, e1 = int(win_start[g]), int(win_start[g + 1])
            k = e1 - e0
            if k == 0:
                continue
            base = w * T * 128
            j = np.arange(k)
            src_idx[base + j] = src_s[e0:e1]
            dloc = dst_s[e0:e1] - (c * npc + w * WIN)
            assert (dloc >= 0).all() and (dloc < WIN).all()
            S[j % 128, w * T + j // 128, dloc] = 1.0
        ST = np.ascontiguousarray(S.transpose(2, 1, 0))  # [dloc, tile, e]
        per_core.append({
            "srcA_w": _wrap_idx(_rowmap_A(src_idx), epc),
            "srcB_w": _wrap_idx(_rowmap_B(src_idx), epc),
            "s_mat": S.astype(S_NP),
            "st_mat": ST.astype(S_NP),
            "xT_own": None,
        })

    f32 = np.float32
    f16 = np.float16
    x = np.asarray(x, f32)
    x_pad = np.zeros((n_pad, IN_DIM), f32)
    x_pad[:n_nodes] = x
    xT_aug = np.concatenate([x_pad.T, np.ones((1, n_pad), f32)], 0)  # [3, n_pad]
    for c in range(ncores):
        per_core[c]["xT_own"] = np.ascontiguousarray(
            xT_aug[:, c * npc:(c + 1) * npc])
        per_core[c]["xTo16"] = per_core[c]["xT_own"].astype(f16)

    W_emb_aug = np.concatenate([np.asarray(W_emb, f32),
                                np.asarray(b_emb, f32)[None, :]], 0)  # [3, 32]

    W2p = np.asarray(W2, f32).reshape(64, D, D).transpose(0, 2, 1).reshape(64, D * D)
    b2p = np.asarray(b2, f32).reshape(D, D).T.reshape(D * D)
    W2pa = np.concatenate([W2p, b2p[None, :]], 0).astype(f16)  # [65, 1024]

    shared = {
        "xT_aug": xT_aug,
        "xT16": xT_aug.astype(f16),
        "w_emb": W_emb_aug,
        "w_emb16": W_emb_aug.astype(f16),
        "w_theta": np.asarray(W_theta, f32).astype(f16),
        "w_phi": np.asarray(W_phi, f32).astype(f16),
        "w1": np.asarray(W1, f32).astype(f16),
        "w2pa": W2pa,
        "w_ih": np.asarray(W_ih, f32),
        "w_hh": np.asarray(W_hh, f32),
        "b_tp": (np.asarray(b_theta, f32) + np.asarray(b_phi, f32))[:, None],
        "b1c": np.asarray(b1, f32)[:, None],
        "b_r": (np.asarray(b_ih, f32)[0:D] + np.asarray(b_hh, f32)[0:D])[:, None],
        "b_z": (np.asarray(b_ih, f32)[D:2 * D] + np.asarray(b_hh, f32)[D:2 * D])[:, None],
        "b_in": np.asarray(b_ih, f32)[2 * D:3 * D][:, None],
        "b_hn": np.asarray(b_hh, f32)[2 * D:3 * D][:, None],
        "idm": np.eye(128, dtype=f32),
        "idm16": np.eye(128).astype(f16),
    }
    meta = dict(T=T, npc=npc, ncores=ncores, n_pad=n_pad, wpc=wpc,
                ntpc=ntpc, epc=epc, steps=STEPS)
    return shared, per_core, meta


# --------------------------------------------------------------------------- #
# device kernel builder
# --------------------------------------------------------------------------- #

def _bcast_mid(ap_base, count):
    aps = [list(p) for p in ap_base.ap]
    new = aps[:-1] + [[0, count]] + [aps[-1]]
    return bass.AP(ap_base.tensor, ap_base.offset, new)


def build_nc(meta):
    T = meta["T"]; npc = meta["npc"]; ncores = meta["ncores"]
    n_pad = meta["n_pad"]; wpc = meta["wpc"]; ntpc = meta["ntpc"]
    epc = meta["epc"]; steps = meta["steps"]
    tpch = ntpc // NCH           # tiles per gather chunk
    epch = tpch * 128            # edges per gather chunk
    assert epch % 16 == 0
    steps_exec = int(os.environ.get("K_STEPS", steps))
    no_a2 = bool(int(os.environ.get("K_NOA2", "0")))
    no_trig = bool(int(os.environ.get("K_NOTRIG", "0")))
    no_sig = bool(int(os.environ.get("K_NOSIG", "0")))
    no_touch = bool(int(os.environ.get("K_NOTOUCH", "0")))

    nc = bacc.Bacc("TRN2", target_bir_lowering=False, debug=False,
                   enable_asserts=False, num_devices=ncores,
                   num_swdge_queues=4)
    global _DBG_NC
    _DBG_NC = nc

    # ---- I/O tensors ----
    t_xT = nc.dram_tensor("xT_aug", [IN_DIM + 1, n_pad], F32, kind="ExternalInput")
    t_xT16 = nc.dram_tensor("xT16", [IN_DIM + 1, n_pad], F16, kind="ExternalInput")
    t_xTo = nc.dram_tensor("xT_own", [IN_DIM + 1, npc], F32, kind="ExternalInput")
    t_xTo16 = nc.dram_tensor("xTo16", [IN_DIM + 1, npc], F16, kind="ExternalInput")
    t_wemb = nc.dram_tensor("w_emb", [IN_DIM + 1, D], F32, kind="ExternalInput")
    t_wemb16 = nc.dram_tensor("w_emb16", [IN_DIM + 1, D], F16, kind="ExternalInput")
    t_wth = nc.dram_tensor("w_theta", [D, D], F16, kind="ExternalInput")
    t_wph = nc.dram_tensor("w_phi", [D, D], F16, kind="ExternalInput")
    t_w1 = nc.dram_tensor("w1", [D, 64], F16, kind="ExternalInput")
    t_w2 = nc.dram_tensor("w2pa", [65, 1024], F16, kind="ExternalInput")
    t_wih = nc.dram_tensor("w_ih", [D, 3 * D], F32, kind="ExternalInput")
    t_whh = nc.dram_tensor("w_hh", [D, 3 * D], F32, kind="ExternalInput")
    t_btp = nc.dram_tensor("b_tp", [D, 1], F32, kind="ExternalInput")
    t_b1c = nc.dram_tensor("b1c", [64, 1], F32, kind="ExternalInput")
    t_br = nc.dram_tensor("b_r", [D, 1], F32, kind="ExternalInput")
    t_bz = nc.dram_tensor("b_z", [D, 1], F32, kind="ExternalInput")
    t_bin = nc.dram_tensor("b_in", [D, 1], F32, kind="ExternalInput")
    t_bhn = nc.dram_tensor("b_hn", [D, 1], F32, kind="ExternalInput")
    t_idm = nc.dram_tensor("idm", [128, 128], F32, kind="ExternalInput")
    t_idm16 = nc.dram_tensor("idm16", [128, 128], F16, kind="ExternalInput")
    t_srcA = nc.dram_tensor("srcA_w", [128, epc // 16], I16, kind="ExternalInput")
    t_srcB = nc.dram_tensor("srcB_w", [128, epc // 16], I16, kind="ExternalInput")
    t_smat = nc.dram_tensor("s_mat", [128, ntpc, 128], F8, kind="ExternalInput")
    t_stmat = nc.dram_tensor("st_mat", [128, ntpc, 128], F8, kind="ExternalInput")
    t_out = nc.dram_tensor("out_h", [npc, D], F32, kind="ExternalOutput")

    sem_q = [nc.alloc_semaphore(f"gsem_q{q}") for q in range(1, NCH + 1)]
    psem = nc.alloc_semaphore("gprep_sem")

    with tile.TileContext(nc) as tc:
        with tc.tile_pool(name="dram", bufs=1, space="DRAM") as dpool, \
             tc.tile_pool(name="const", bufs=1) as cpool:
            we_dram = dpool.tile([epc, 1024], F16, name="we_dram")
            h_full0 = dpool.tile([n_pad, 128], F16, name="h_full0")
            h_fulls = [dpool.tile([n_pad, 128], F16, addr_space="Shared",
                                  name=f"h_full{s}") for s in (1, 2)]
            cc_ins = [dpool.tile([npc, 128], F16, name=f"cc_in{s}")
                      for s in range(steps - 1)]

            # resident constants
            idm = cpool.tile([128, 128], F32, name="idm")
            nc.sync.dma_start(idm[:], t_idm.ap())
            idm16 = cpool.tile([128, 128], F16, name="idm16")
            nc.sync.dma_start(idm16[:], t_idm16.ap())
            touch = cpool.tile([1, 2], F16, name="touch")
            tsem = nc.alloc_semaphore("touch_sem")
            S_sb = cpool.tile([128, ntpc * 128], F8, name="S_sb")
            nc.sync.dma_start(S_sb[:], t_smat.ap().rearrange("p t e -> p (t e)"))
            iA = cpool.tile([128, epc // 16], I16, name="iA")
            nc.sync.dma_start(iA[:], t_srcA.ap())
            iB = cpool.tile([128, epc // 16], I16, name="iB")
            nc.sync.dma_start(iB[:], t_srcB.ap())

            def load_const(t, shape, dtype, name):
                s = cpool.tile(shape, dtype, name=name)
                nc.sync.dma_start(s[:], t.ap())
                return s

            xTo_sb = load_const(t_xTo, [IN_DIM + 1, npc], F32, "xTo_sb")
            wemb_sb = load_const(t_wemb, [IN_DIM + 1, D], F32, "wemb_sb")
            wemb16_sb = load_const(t_wemb16, [IN_DIM + 1, D], F16, "wemb16_sb")
            wth_sb = load_const(t_wth, [D, D], F16, "wth_sb")
            wph_sb = load_const(t_wph, [D, D], F16, "wph_sb")
            w1_sb = load_const(t_w1, [D, 64], F16, "w1_sb")
            w2_sb = load_const(t_w2, [65, 1024], F16, "w2_sb")
            wih_sb = load_const(t_wih, [D, 3 * D], F32, "wih_sb")
            whh_sb = load_const(t_whh, [D, 3 * D], F32, "whh_sb")
            btp_sb = load_const(t_btp, [D, 1], F32, "btp_sb")
            b1c_sb = load_const(t_b1c, [64, 1], F32, "b1c_sb")
            br_sb = load_const(t_br, [D, 1], F32, "br_sb")
            bz_sb = load_const(t_bz, [D, 1], F32, "bz_sb")
            bin_sb = load_const(t_bin, [D, 1], F32, "bin_sb")
            bhn_sb = load_const(t_bhn, [D, 1], F32, "bhn_sb")

            # GRU state (transposed layout), ping-pong across steps
            h_bufs = [cpool.tile([D, npc], F32, name=f"hT{i}") for i in range(2)]
            # gathered h[src] (shared by A2 and each step's DVE stage)
            G = cpool.tile([128, ntpc, 128], F16, name="G")

            def prep_gather(idx_sb, h_src_ap):
                for c in range(NCH):
                    i = nc.gpsimd.dma_gather(
                        G[:, c * tpch:(c + 1) * tpch, :], h_src_ap,
                        idx_sb[:, c * (epch // 16):(c + 1) * (epch // 16)],
                        epch, epch, 128,
                        transpose=False, single_packet=False,
                        prepare_only=True, sem=sem_q[c], queue_num=1 + c)
                    i.then_inc(psem, 1)

            def trigger_gather(round_no, src_ap, prep_args=None):
                # proven shape: preps + trigger + completion waits in ONE
                # critical (count=1, validated on HW). Desc-gen sits on the
                # critical path for now.
                with tc.tile_critical():
                    prep_gather(*prep_args)
                    nc.gpsimd.wait_ge(psem, NCH * round_no)
                    for c in range(NCH):
                        nc.gpsimd.trigger_dma(count=1, queue_num=1 + c)
                    for c in range(NCH):
                        nc.gpsimd.wait_ge(sem_q[c], 16 * round_no)

            late_prep = int(os.environ.get("K_LATEPREP", "0"))


            # ---------------- A1 + A2 scoped pool ----------------
            _apool_cm = tc.tile_pool(name="aph", bufs=1)
            apool = _apool_cm.__enter__()
            h0st = apool.tile([128, 80, 128], F16, name="h0st")
            xT16 = apool.tile([IN_DIM + 1, n_pad], F16, name="xT16")
            nc.sync.dma_start(xT16[:], t_xT16.ap())
            # ---------------- A1: h0 ----------------
            with tc.tile_pool(name="pA1", bufs=2, space="PSUM") as pp1:
                nc.vector.memset(h0st[:], 0.0)
                for ch in range(80):
                    ps = pp1.tile([128, D], F32, tag="psh0")
                    nc.tensor.matmul(ps[:], lhsT=xT16[:, ch * 128:(ch + 1) * 128],
                                     rhs=wemb16_sb[:], start=True, stop=True)
                    nc.scalar.copy(h0st[:, ch, 0:D], ps[:])
                # one contiguous-per-partition descriptor per partition
                if not bool(int(os.environ.get("K_NOSTORE", "0"))):
                    nc.sync.dma_start(
                        h_full0[:].rearrange("(p t) f -> p (t f)", p=128), h0st[:])
                # own-window node-major h0 (dst-side stationaries for A2)
                xTo16_sb = apool.tile([IN_DIM + 1, npc], F16, name="xTo16_sb")
                nc.sync.dma_start(xTo16_sb[:], t_xTo16.ap())
                h0own = apool.tile([128, wpc, D], F16, name="h0own")
                for w in range(wpc):
                    ps = pp1.tile([128, D], F32, tag="psh0")
                    nc.tensor.matmul(ps[:], lhsT=xTo16_sb[:, w * 128:(w + 1) * 128],
                                     rhs=wemb16_sb[:], start=True, stop=True)
                    nc.scalar.copy(h0own[:, w, :], ps[:])
                # own-range h0 transposed (fp32 GRU state)
                for c0 in range(0, npc, 512):
                    cn = min(512, npc - c0)
                    ps = pp1.tile([D, 512], F32, tag="pshT")
                    nc.tensor.matmul(ps[:, 0:cn], lhsT=wemb_sb[:],
                                     rhs=xTo_sb[:, c0:c0 + cn], start=True, stop=True)
                    nc.vector.tensor_copy(h_bufs[0][:, c0:c0 + cn], ps[:, 0:cn])

            # gather round 1: h0[src] -> G (waits for h_full0 via prep deps)
            if not no_trig:
                trigger_gather(1, h_full0[0:1, 0:2],
                               prep_args=(iA, h_full0[:, :]))

            # ---------------- A2: edge MLP -> We ----------------
            with tc.tile_pool(name="pHD", bufs=1, space="PSUM") as phd, \
                 tc.tile_pool(name="pHE", bufs=1, space="PSUM") as phe, \
                 tc.tile_pool(name="pW", bufs=2, space="PSUM") as pw, \
                 tc.tile_pool(name="sA2", bufs=3) as sp2, \
                 tc.tile_pool(name="sST", bufs=1) as spst, \
                 tc.tile_pool(name="sWt", bufs=3) as spw:
                ST_sb = spst.tile([128, ntpc * 128], F8, name="ST_sb")
                nc.sync.dma_start(ST_sb[:],
                                  t_stmat.ap().rearrange("p t e -> p (t e)"))
                evac_flip = 0
                for t0 in (range(0, ntpc, 4) if not no_a2 else []):  # 4 tiles = 512 edges
                    tn = 4
                    en = tn * 128
                    # hdT via one-hot ST matmuls (dst is window-local)
                    pshd = phd.tile([D, 512], F32, tag="pshd")
                    for j in range(tn):
                        gt = t0 + j
                        nc.tensor.matmul(
                            pshd[:, j * 128:(j + 1) * 128],
                            lhsT=h0own[:, gt // T, :],
                            rhs=ST_sb[:, gt * 128:(gt + 1) * 128],
                            start=True, stop=True)
                    # hsT for the 4 tiles via one batched PE transpose:
                    # in [128e, (4t, 32d)] -> psum [(4t, 32d), 128e]
                    Gc = sp2.tile([128, 128], F16, tag="Gc")
                    nc.vector.tensor_copy(
                        Gc[:].rearrange("p (t d) -> p t d", t=4),
                        G[:, t0:t0 + 4, 0:D])
                    pshs = phd.tile([128, 128], F16, tag="pshs")
                    nc.tensor.transpose(pshs[:], Gc[:], idm16[:])
                    hsT = sp2.tile([D, 512], F16, tag="hsT")
                    for j in range(tn):
                        nc.scalar.copy(hsT[:, j * 128:(j + 1) * 128],
                                       pshs[j * D:(j + 1) * D, :])
                    dT = sp2.tile([D, 512], F16, tag="dT")
                    nc.vector.tensor_sub(dT[:, 0:en], pshd[:, 0:en], hsT[:, 0:en])
                    # he = relu(Wth dT + Wph hsT + b)
                    psh = phe.tile([D, 512], F32, tag="pshe")
                    nc.tensor.matmul(psh[:, 0:en], lhsT=wth_sb[:],
                                     rhs=dT[:, 0:en], start=True, stop=False)
                    nc.tensor.matmul(psh[:, 0:en], lhsT=wph_sb[:],
                                     rhs=hsT[:, 0:en], start=False, stop=True)
                    he = sp2.tile([D, 512], F16, tag="he")
                    nc.scalar.activation(he[:, 0:en], psh[:, 0:en], AF.Relu,
                                         bias=btp_sb[:])
                    psg = phe.tile([64, 512], F32, tag="psg")
                    nc.tensor.matmul(psg[:, 0:en], lhsT=w1_sb[:], rhs=he[:, 0:en],
                                     start=True, stop=True)
                    ga = sp2.tile([65, 512], F16, tag="ga")
                    nc.vector.memset(ga[64:65, 0:en], 1.0)
                    nc.scalar.activation(ga[0:64, 0:en], psg[:, 0:en], AF.Relu,
                                         bias=b1c_sb[:])
                    for j in range(tn):
                        gt = t0 + j
                        s0 = j * 128
                        pw0 = pw.tile([128, 512], F32, tag="psw0")
                        pw1 = pw.tile([128, 512], F32, tag="psw1")
                        nc.tensor.matmul(pw0[:], lhsT=ga[:, s0:s0 + 128],
                                         rhs=w2_sb[:, 0:512], start=True, stop=True)
                        nc.tensor.matmul(pw1[:], lhsT=ga[:, s0:s0 + 128],
                                         rhs=w2_sb[:, 512:1024], start=True, stop=True)
                        wt = spw.tile([128, 1024], F16, tag="wt")
                        if evac_flip == 0:
                            nc.scalar.copy(wt[:, 0:512], pw0[:])
                            nc.vector.tensor_copy(wt[:, 512:1024], pw1[:])
                        else:
                            nc.vector.tensor_copy(wt[:, 0:512], pw0[:])
                            nc.scalar.copy(wt[:, 512:1024], pw1[:])
                        evac_flip ^= 1
                        nc.sync.dma_start(
                            we_dram[gt * 128:(gt + 1) * 128, :], wt[:])

            _apool_cm.__exit__(None, None, None)

            # ---------------- Phase B: message passing steps ----------------
            we_view = we_dram[:].rearrange("(t p) f -> p t f", p=128)
            with tc.tile_pool(name="sWq", bufs=2) as swq, \
                 tc.tile_pool(name="sPr", bufs=2) as spr, \
                 tc.tile_pool(name="sP16", bufs=2) as sp16, \
                 tc.tile_pool(name="sWin", bufs=2) as swin, \
                 tc.tile_pool(name="sGru", bufs=1) as sgru, \
                 tc.tile_pool(name="pA", bufs=2, space="PSUM") as ppa, \
                 tc.tile_pool(name="pT", bufs=1, space="PSUM") as ppt, \
                 tc.tile_pool(name="pG", bufs=1, space="PSUM") as ppg:
                for step in range(steps_exec):
                    h_cur = h_bufs[step % 2]
                    h_new = h_bufs[(step + 1) % 2]

                    if step > 0:
                        trigger_gather(step + 1, h_fulls[step - 1][0:1, 0:2],
                                       prep_args=(iB, h_fulls[step - 1][:, :]))

                    aT = sgru.tile([D, npc], F32, tag="aT")
                    psa = None
                    for q0 in range(0, ntpc, 4):
                        k = min(4, ntpc - q0)
                        wq = swq.tile([128, 4, 1024], F16, tag="wq")
                        nc.sync.dma_start(wq[:, 0:k, :], we_view[:, q0:q0 + k, :])
                        prod = spr.tile([128, 4, D, D], F16, tag="prod")
                        base = G[:, q0:q0 + k, 0:D]
                        in1 = _bcast_mid(base, D)
                        nc.vector.tensor_tensor(
                            prod[:, 0:k, :, :],
                            wq[:, 0:k, :].rearrange("p t (o i) -> p t o i", o=D),
                            in1, op=OP.mult)
                        p16 = sp16.tile([128, 4, D, 16], F16, tag="p16")
                        nc.vector.tensor_tensor(
                            p16[:, 0:k, :, :], prod[:, 0:k, :, 0:16],
                            prod[:, 0:k, :, 16:32], op=OP.add)
                        for j in range(k):
                            gt = q0 + j
                            w = gt // T
                            tloc = gt % T
                            if tloc == 0:
                                psa = ppa.tile([128, 512], F32, tag="psa")
                            nc.tensor.matmul(
                                psa[:], lhsT=S_sb[:, gt * 128:(gt + 1) * 128],
                                rhs=p16[:, j, :, :],
                                start=(tloc == 0), stop=(tloc == T - 1))
                            if tloc == T - 1:
                                aw = swin.tile([128, D, 16], F32, tag="aw")
                                nc.scalar.copy(
                                    aw[:], psa[:].rearrange("p (o i) -> p o i", o=D))
                                t8 = swin.tile([128, D, 8], F32, tag="t8")
                                nc.vector.tensor_tensor(t8[:], aw[:, :, 0:8],
                                                        aw[:, :, 8:16], op=OP.add)
                                t4 = swin.tile([128, D, 4], F32, tag="t4")
                                nc.vector.tensor_tensor(t4[:], t8[:, :, 0:4],
                                                        t8[:, :, 4:8], op=OP.add)
                                t2 = swin.tile([128, D, 2], F32, tag="t2")
                                nc.vector.tensor_tensor(t2[:], t4[:, :, 0:2],
                                                        t4[:, :, 2:4], op=OP.add)
                                t1 = swin.tile([128, D], F32, tag="t1")
                                nc.vector.tensor_tensor(t1[:], t2[:, :, 0],
                                                        t2[:, :, 1], op=OP.add)
                                pst = ppt.tile([D, 128], F32, tag="pst")
                                nc.tensor.transpose(pst[:], t1[:], idm[:])
                                nc.vector.tensor_copy(
                                    aT[:, w * 128:(w + 1) * 128], pst[:])

                    # ---- GRU (transposed layout) ----
                    for c0 in range(0, npc, 512):
                        cn = min(512, npc - c0)
                        cs = slice(c0, c0 + cn)
                        pgi = ppg.tile([3 * D, 512], F32, tag="pgi")
                        nc.tensor.matmul(pgi[:, 0:cn], lhsT=wih_sb[:],
                                         rhs=aT[:, cs], start=True, stop=True)
                        pgh = ppg.tile([3 * D, 512], F32, tag="pgh")
                        nc.tensor.matmul(pgh[:, 0:cn], lhsT=whh_sb[:],
                                         rhs=h_cur[:, cs], start=True, stop=True)
                        gh_sb = sgru.tile([3 * D, 512], F32, tag="gh_sb")
                        nc.scalar.copy(gh_sb[:, 0:cn], pgh[:, 0:cn])
                        tr = sgru.tile([D, 512], F32, tag="tr")
                        nc.vector.tensor_add(tr[:, 0:cn], pgi[0:D, 0:cn],
                                             gh_sb[0:D, 0:cn])
                        r = sgru.tile([D, 512], F32, tag="r")
                        nc.scalar.activation(r[:, 0:cn], tr[:, 0:cn], AF.Sigmoid,
                                             bias=br_sb[:])
                        tz = sgru.tile([D, 512], F32, tag="tz")
                        nc.vector.tensor_add(tz[:, 0:cn], pgi[D:2 * D, 0:cn],
                                             gh_sb[D:2 * D, 0:cn])
                        z = sgru.tile([D, 512], F32, tag="z")
                        nc.scalar.activation(z[:, 0:cn], tz[:, 0:cn], AF.Sigmoid,
                                             bias=bz_sb[:])
                        hnb = sgru.tile([D, 512], F32, tag="hnb")
                        nc.vector.tensor_scalar_add(hnb[:, 0:cn],
                                                    gh_sb[2 * D:3 * D, 0:cn],
                                                    bhn_sb[:])
                        rhn = sgru.tile([D, 512], F32, tag="rhn")
                        nc.vector.tensor_mul(rhn[:, 0:cn], r[:, 0:cn], hnb[:, 0:cn])
                        tn_ = sgru.tile([D, 512], F32, tag="tn_")
                        nc.vector.tensor_add(tn_[:, 0:cn], rhn[:, 0:cn],
                                             pgi[2 * D:3 * D, 0:cn])
                        ngate = sgru.tile([D, 512], F32, tag="ngate")
                        nc.scalar.activation(ngate[:, 0:cn], tn_[:, 0:cn], AF.Tanh,
                                             bias=bin_sb[:])
                        hmn = sgru.tile([D, 512], F32, tag="hmn")
                        nc.vector.tensor_sub(hmn[:, 0:cn], h_cur[:, cs],
                                             ngate[:, 0:cn])
                        zh = sgru.tile([D, 512], F32, tag="zh")
                        nc.vector.tensor_mul(zh[:, 0:cn], z[:, 0:cn], hmn[:, 0:cn])
                        nc.vector.tensor_add(h_new[:, cs], ngate[:, 0:cn],
                                             zh[:, 0:cn])

                    # ---- write h out; AllGather (except after last step) ----
                    if step < steps_exec - 1:
                        hst = sgru.tile([128, wpc, 128], F16, tag="hst")
                        if step == 0:
                            nc.vector.memset(hst[:], 0.0)
                        for w in range(wpc):
                            ps2 = ppt.tile([128, D], F32, tag="ps2")
                            nc.tensor.transpose(ps2[:],
                                                h_new[:, w * 128:(w + 1) * 128],
                                                idm[0:D, 0:D])
                            nc.scalar.copy(hst[:, w, 0:D], ps2[:])
                        nc.sync.dma_start(
                            cc_ins[step][:].rearrange("(p w) f -> p (w f)", p=128),
                            hst[:])
                        nc.gpsimd.collective_compute(
                            "AllGather", OP.bypass,
                            replica_groups=[list(range(ncores))],
                            ins=[cc_ins[step][:].opt()],
                            outs=[h_fulls[step][:].opt()])
                    else:
                        ost = sgru.tile([128, wpc, D], F32, tag="ost")
                        for w in range(wpc):
                            ps2 = ppt.tile([128, D], F32, tag="ps2")
                            nc.tensor.transpose(ps2[:],
                                                h_new[:, w * 128:(w + 1) * 128],
                                                idm[0:D, 0:D])
                            nc.scalar.copy(ost[:, w, :], ps2[:])
                        nc.sync.dma_start(
                            t_out.ap().rearrange("(w p) d -> p w d", p=128), ost[:])

        if steps_exec == 0:
            with tc.tile_pool(name="sZ", bufs=1) as sz:
                zst = sz.tile([128, wpc, D], F32, name="zst")
                nc.vector.memset(zst[:], 0.0)
                nc.sync.dma_start(
                    t_out.ap().rearrange("(w p) d -> p w d", p=128), zst[:])
    nc.compile()
    return nc


# --------------------------------------------------------------------------- #
# entry point
# --------------------------------------------------------------------------- #

def run(inputs, n_nodes=N_NODES, npc=NPC, **spmd_kwargs):
    global LAST_RESULT, LAST_META
    shared, per_core, meta = host_prep(**inputs, n_nodes=n_nodes, npc=npc)
    LAST_META = meta
    nc = build_nc(meta)
    in_maps = [dict(shared, **pc) for pc in per_core]
    res = bass_utils.run_bass_kernel_spmd(
        nc, in_maps, core_ids=list(range(meta["ncores"])), **spmd_kwargs)
    LAST_RESULT = res
    out = np.concatenate([res.results[c]["out_h"] for c in range(meta["ncores"])], 0)
    return np.ascontiguousarray(out[:n_nodes]).astype(np.float32)


def kernel(**inputs):
    return run(inputs)


# revision 27
# speedup vs baseline: 1.5240x; 1.2202x over previous
"""Trainium2 Bass kernel for nn_GNN_GRU_83519934038653 (GatedGraphConv-style GNN).

Strategy (8 NeuronCores, SPMD, one NEFF):
  - Host: sort edges by dst, shard by dst node-range (1280 nodes/core), build
    int16 gather index tables + per-tile one-hot scatter matrices S (and their
    per-tile transposes ST for the dst-side select), permute weights.
  - h tables in DRAM use a host-chosen row permutation so every SBUF->DRAM h
    write is one contiguous descriptor per partition; the gather index tables
    absorb the permutation.
  - Gathers run as prepare_only descriptor generation on SWDGE queues 1-3
    (hidden under compute) + trigger_dma when the source table is ready;
    pre-staged descriptors drain at ~250 GB/s vs ~40 GB/s gen-paced.
  - A2: one gather of h0[src] shared with step 1; h0[dst] needs no gather at
    all (dst is window-local: one-hot ST matmuls against h0 windows).
  - Phase B x3: wq = We chunk DMA; DVE multiply (h broadcast over o);
    one tree-add level; PE scatter-matmul with one-hot S; per-window tree;
    transposed GRU; AllGather h (except last step).
"""

import os
import sys

for _p in ("/opt/trn_rl_repo", "/root/.axon_site/_ro/trn_rl_repo"):
    if os.path.isdir(_p) and _p not in sys.path:
        sys.path.insert(0, _p)

import numpy as np

import concourse.bass as bass
import concourse.bacc as bacc
import concourse.mybir as mybir
import concourse.tile as tile
import concourse.bass_utils as bass_utils

F32 = mybir.dt.float32
F16 = mybir.dt.float16
F8 = mybir.dt.float8e4
BF16 = mybir.dt.bfloat16
I16 = mybir.dt.int16
AF = mybir.ActivationFunctionType
OP = mybir.AluOpType

N_NODES = 10000
N_EDGES = 160000
D = 32
IN_DIM = 2
STEPS = 3
CORES = 8
NPC = 1280   # padded nodes per core; 8*1280 = 10240
WIN = 128    # scatter window (nodes per PSUM accumulation window)
NCH = 3      # gather chunks (SWDGE queues 1..NCH)

S_NP = mybir.dt.np(F8)
import ml_dtypes
_BF16_NP = ml_dtypes.bfloat16

LAST_RESULT = None
LAST_META = None


# --------------------------------------------------------------------------- #
# host-side preparation
# --------------------------------------------------------------------------- #

def _wrap_idx(idx, epc):
    w = idx.reshape(epc // 16, 16).T.astype(np.int16)
    return np.tile(w, (8, 1)).copy()


def _rowmap_A(n):
    """h_full0 row of node n: h0st[p, w] holds node w*128+p, stored p-major."""
    return (n % 128) * 80 + n // 128


def _rowmap_B(n):
    """h_full(step) row of node n after AllGather of p-major cc_in shards."""
    c = n // NPC
    loc = n % NPC
    return c * NPC + (loc % 128) * (NPC // 128) + loc // 128


def host_prep(x, src, dst, W_emb, b_emb, W_theta, b_theta, W_phi, b_phi,
              W1, b1, W2, b2, W_ih, b_ih, W_hh, b_hh,
              n_nodes=N_NODES, npc=NPC, ncores=CORES):
    n_pad = npc * ncores
    wpc = npc // WIN
    nwin = ncores * wpc

    src = np.asarray(src).astype(np.int64)
    dst = np.asarray(dst).astype(np.int64)
    order = np.argsort(dst, kind="stable")
    src_s = src[order]
    dst_s = dst[order]

    win_of_edge = dst_s // WIN
    counts = np.bincount(win_of_edge, minlength=nwin)
    T = max(1, int(np.ceil(counts.max() / 128)))
    ntpc = wpc * T
    epc = ntpc * 128
    assert epc % (16 * NCH) == 0 and ntpc % NCH == 0

    win_start = np.concatenate([[0], np.cumsum(counts)])

    per_core = []
    spread = (np.arange(epc, dtype=np.int64) * 127) % n_pad
    for c in range(ncores):
        src_idx = spread.copy()
        S = np.zeros((128, ntpc, 128), np.float32)
        for w in range(wpc):
            g = c * wpc + w
            e0, e1 = int(win_start[g]), int(win_start[g + 1])
            k = e1 - e0
            if k == 0:
                continue
            base = w * T * 128
            j = np.arange(k)
            src_idx[base + j] = src_s[e0:e1]
            dloc = dst_s[e0:e1] - (c * npc + w * WIN)
            assert (dloc >= 0).all() and (dloc < WIN).all()
            S[j % 128, w * T + j // 128, dloc] = 1.0
        ST = np.ascontiguousarray(S.transpose(2, 1, 0))  # [dloc, tile, e]
        per_core.append({
            "srcA_w": _wrap_idx(_rowmap_A(src_idx), epc),
            "srcB_w": _wrap_idx(_rowmap_B(src_idx), epc),
            "s_mat": S.astype(S_NP),
            "st_mat": ST.astype(S_NP),
            "xT_own": None,
        })

    f32 = np.float32
    f16 = np.float16
    x = np.asarray(x, f32)
    x_pad = np.zeros((n_pad, IN_DIM), f32)
    x_pad[:n_nodes] = x
    xT_aug = np.concatenate([x_pad.T, np.ones((1, n_pad), f32)], 0)  # [3, n_pad]
    for c in range(ncores):
        per_core[c]["xT_own"] = np.ascontiguousarray(
            xT_aug[:, c * npc:(c + 1) * npc])
        per_core[c]["xTo16"] = per_core[c]["xT_own"].astype(f16)

    W_emb_aug = np.concatenate([np.asarray(W_emb, f32),
                                np.asarray(b_emb, f32)[None, :]], 0)  # [3, 32]

    W2p = np.asarray(W2, f32).reshape(64, D, D).transpose(0, 2, 1).reshape(64, D * D)
    b2p = np.asarray(b2, f32).reshape(D, D).T.reshape(D * D)
    W2pa = np.concatenate([W2p, b2p[None, :]], 0).astype(f16)  # [65, 1024]

    shared = {
        "xT_aug": xT_aug,
        "xT16": xT_aug.astype(f16),
        "w_emb": W_emb_aug,
        "w_emb16": W_emb_aug.astype(f16),
        "w_theta": np.asarray(W_theta, f32).astype(f16),
        "w_phi": np.asarray(W_phi, f32).astype(f16),
        "w1": np.asarray(W1, f32).astype(f16),
        "w2pa": W2pa.astype(np.float32).astype(_BF16_NP),
        "w_ih": np.asarray(W_ih, f32),
        "w_hh": np.asarray(W_hh, f32),
        "b_tp": (np.asarray(b_theta, f32) + np.asarray(b_phi, f32))[:, None],
        "b1c": np.asarray(b1, f32)[:, None],
        "b_r": (np.asarray(b_ih, f32)[0:D] + np.asarray(b_hh, f32)[0:D])[:, None],
        "b_z": (np.asarray(b_ih, f32)[D:2 * D] + np.asarray(b_hh, f32)[D:2 * D])[:, None],
        "b_in": np.asarray(b_ih, f32)[2 * D:3 * D][:, None],
        "b_hn": np.asarray(b_hh, f32)[2 * D:3 * D][:, None],
        "idm": np.eye(128, dtype=f32),
        "idm16": np.eye(128).astype(f16),
    }
    meta = dict(T=T, npc=npc, ncores=ncores, n_pad=n_pad, wpc=wpc,
                ntpc=ntpc, epc=epc, steps=STEPS)
    return shared, per_core, meta


# --------------------------------------------------------------------------- #
# device kernel builder
# --------------------------------------------------------------------------- #

def _bcast_mid(ap_base, count):
    aps = [list(p) for p in ap_base.ap]
    new = aps[:-1] + [[0, count]] + [aps[-1]]
    return bass.AP(ap_base.tensor, ap_base.offset, new)


def build_nc(meta):
    T = meta["T"]; npc = meta["npc"]; ncores = meta["ncores"]
    n_pad = meta["n_pad"]; wpc = meta["wpc"]; ntpc = meta["ntpc"]
    epc = meta["epc"]; steps = meta["steps"]
    tpch = ntpc // NCH           # tiles per gather chunk
    epch = tpch * 128            # edges per gather chunk
    assert epch % 16 == 0
    steps_exec = int(os.environ.get("K_STEPS", steps))
    no_a2 = bool(int(os.environ.get("K_NOA2", "0")))
    no_trig = bool(int(os.environ.get("K_NOTRIG", "0")))
    no_sig = bool(int(os.environ.get("K_NOSIG", "0")))
    no_touch = bool(int(os.environ.get("K_NOTOUCH", "0")))

    nc = bacc.Bacc("TRN2", target_bir_lowering=False, debug=False,
                   enable_asserts=False, num_devices=ncores,
                   num_swdge_queues=4)
    global _DBG_NC
    _DBG_NC = nc

    # ---- I/O tensors ----
    t_xT = nc.dram_tensor("xT_aug", [IN_DIM + 1, n_pad], F32, kind="ExternalInput")
    t_xT16 = nc.dram_tensor("xT16", [IN_DIM + 1, n_pad], F16, kind="ExternalInput")
    t_xTo = nc.dram_tensor("xT_own", [IN_DIM + 1, npc], F32, kind="ExternalInput")
    t_xTo16 = nc.dram_tensor("xTo16", [IN_DIM + 1, npc], F16, kind="ExternalInput")
    t_wemb = nc.dram_tensor("w_emb", [IN_DIM + 1, D], F32, kind="ExternalInput")
    t_wemb16 = nc.dram_tensor("w_emb16", [IN_DIM + 1, D], F16, kind="ExternalInput")
    t_wth = nc.dram_tensor("w_theta", [D, D], F16, kind="ExternalInput")
    t_wph = nc.dram_tensor("w_phi", [D, D], F16, kind="ExternalInput")
    t_w1 = nc.dram_tensor("w1", [D, 64], F16, kind="ExternalInput")
    t_w2 = nc.dram_tensor("w2pa", [65, 1024], BF16, kind="ExternalInput")
    t_wih = nc.dram_tensor("w_ih", [D, 3 * D], F32, kind="ExternalInput")
    t_whh = nc.dram_tensor("w_hh", [D, 3 * D], F32, kind="ExternalInput")
    t_btp = nc.dram_tensor("b_tp", [D, 1], F32, kind="ExternalInput")
    t_b1c = nc.dram_tensor("b1c", [64, 1], F32, kind="ExternalInput")
    t_br = nc.dram_tensor("b_r", [D, 1], F32, kind="ExternalInput")
    t_bz = nc.dram_tensor("b_z", [D, 1], F32, kind="ExternalInput")
    t_bin = nc.dram_tensor("b_in", [D, 1], F32, kind="ExternalInput")
    t_bhn = nc.dram_tensor("b_hn", [D, 1], F32, kind="ExternalInput")
    t_idm = nc.dram_tensor("idm", [128, 128], F32, kind="ExternalInput")
    t_idm16 = nc.dram_tensor("idm16", [128, 128], F16, kind="ExternalInput")
    t_srcA = nc.dram_tensor("srcA_w", [128, epc // 16], I16, kind="ExternalInput")
    t_srcB = nc.dram_tensor("srcB_w", [128, epc // 16], I16, kind="ExternalInput")
    t_smat = nc.dram_tensor("s_mat", [128, ntpc, 128], F8, kind="ExternalInput")
    t_stmat = nc.dram_tensor("st_mat", [128, ntpc, 128], F8, kind="ExternalInput")
    t_out = nc.dram_tensor("out_h", [npc, D], F32, kind="ExternalOutput")

    sem_q = [nc.alloc_semaphore(f"gsem_q{q}") for q in range(1, NCH + 1)]
    psem = nc.alloc_semaphore("gprep_sem")

    with tile.TileContext(nc) as tc:
        with tc.tile_pool(name="dram", bufs=1, space="DRAM") as dpool, \
             tc.tile_pool(name="const", bufs=1) as cpool:
            we_dram = dpool.tile([epc, 1024], F16, name="we_dram")
            h_full0 = dpool.tile([n_pad, 128], F16, name="h_full0")
            h_fulls = [dpool.tile([n_pad, 128], F16, addr_space="Shared",
                                  name=f"h_full{s}") for s in (1, 2)]
            cc_ins = [dpool.tile([npc, 128], F16, name=f"cc_in{s}")
                      for s in range(steps - 1)]

            # resident constants
            idm = cpool.tile([128, 128], F32, name="idm")
            nc.sync.dma_start(idm[:], t_idm.ap())
            idm16 = cpool.tile([128, 128], F16, name="idm16")
            nc.sync.dma_start(idm16[:], t_idm16.ap())
            touch = cpool.tile([1, 2], F16, name="touch")
            tsem = nc.alloc_semaphore("touch_sem")
            S_sb = cpool.tile([128, ntpc * 128], F8, name="S_sb")
            nc.sync.dma_start(S_sb[:], t_smat.ap().rearrange("p t e -> p (t e)"))
            iA = cpool.tile([128, epc // 16], I16, name="iA")
            nc.sync.dma_start(iA[:], t_srcA.ap())
            iB = cpool.tile([128, epc // 16], I16, name="iB")
            nc.sync.dma_start(iB[:], t_srcB.ap())

            def load_const(t, shape, dtype, name):
                s = cpool.tile(shape, dtype, name=name)
                nc.sync.dma_start(s[:], t.ap())
                return s

            xTo_sb = load_const(t_xTo, [IN_DIM + 1, npc], F32, "xTo_sb")
            wemb_sb = load_const(t_wemb, [IN_DIM + 1, D], F32, "wemb_sb")
            wemb16_sb = load_const(t_wemb16, [IN_DIM + 1, D], F16, "wemb16_sb")
            wth_sb = load_const(t_wth, [D, D], F16, "wth_sb")
            wph_sb = load_const(t_wph, [D, D], F16, "wph_sb")
            w1_sb = load_const(t_w1, [D, 64], F16, "w1_sb")
            w2_sb = load_const(t_w2, [65, 1024], BF16, "w2_sb")
            wih_sb = load_const(t_wih, [D, 3 * D], F32, "wih_sb")
            whh_sb = load_const(t_whh, [D, 3 * D], F32, "whh_sb")
            btp_sb = load_const(t_btp, [D, 1], F32, "btp_sb")
            b1c_sb = load_const(t_b1c, [64, 1], F32, "b1c_sb")
            br_sb = load_const(t_br, [D, 1], F32, "br_sb")
            bz_sb = load_const(t_bz, [D, 1], F32, "bz_sb")
            bin_sb = load_const(t_bin, [D, 1], F32, "bin_sb")
            bhn_sb = load_const(t_bhn, [D, 1], F32, "bhn_sb")

            # GRU state (transposed layout), ping-pong across steps
            h_bufs = [cpool.tile([D, npc], F32, name=f"hT{i}") for i in range(2)]
            # gathered h[src] (shared by A2 and each step's DVE stage)
            G = cpool.tile([128, ntpc, 128], F16, name="G")

            def prep_gather(idx_sb, h_src_ap):
                for c in range(NCH):
                    i = nc.gpsimd.dma_gather(
                        G[:, c * tpch:(c + 1) * tpch, :], h_src_ap,
                        idx_sb[:, c * (epch // 16):(c + 1) * (epch // 16)],
                        epch, epch, 128,
                        transpose=False, single_packet=False,
                        prepare_only=True, sem=sem_q[c], queue_num=1 + c)
                    i.then_inc(psem, 1)

            def trigger_gather(round_no, src_ap, prep_args=None):
                # trigger + completion waits in one critical; preps either
                # inline (round 1) or emitted earlier (bare) to hide desc-gen
                with tc.tile_critical():
                    if prep_args is not None:
                        prep_gather(*prep_args)
                    nc.gpsimd.wait_ge(psem, NCH * round_no)
                    for c in range(NCH):
                        nc.gpsimd.trigger_dma(count=1, queue_num=1 + c)
                    for c in range(NCH):
                        nc.gpsimd.wait_ge(sem_q[c], 16 * round_no)

            late_prep = int(os.environ.get("K_LATEPREP", "0"))


            # ---------------- A1 + A2 scoped pool ----------------
            _apool_cm = tc.tile_pool(name="aph", bufs=1)
            apool = _apool_cm.__enter__()
            h0st = apool.tile([128, 80, 128], F16, name="h0st")
            xT16 = apool.tile([IN_DIM + 1, n_pad], F16, name="xT16")
            nc.sync.dma_start(xT16[:], t_xT16.ap())
            # ---------------- A1: h0 ----------------
            with tc.tile_pool(name="pA1", bufs=2, space="PSUM") as pp1:
                nc.vector.memset(h0st[:], 0.0)
                for ch in range(80):
                    ps = pp1.tile([128, D], F32, tag="psh0")
                    nc.tensor.matmul(ps[:], lhsT=xT16[:, ch * 128:(ch + 1) * 128],
                                     rhs=wemb16_sb[:], start=True, stop=True)
                    nc.scalar.copy(h0st[:, ch, 0:D], ps[:])
                # one contiguous-per-partition descriptor per partition
                if not bool(int(os.environ.get("K_NOSTORE", "0"))):
                    nc.sync.dma_start(
                        h_full0[:].rearrange("(p t) f -> p (t f)", p=128), h0st[:])
                # own-window node-major h0 (dst-side stationaries for A2)
                xTo16_sb = apool.tile([IN_DIM + 1, npc], F16, name="xTo16_sb")
                nc.sync.dma_start(xTo16_sb[:], t_xTo16.ap())
                h0own = apool.tile([128, wpc, D], F16, name="h0own")
                for w in range(wpc):
                    ps = pp1.tile([128, D], F32, tag="psh0")
                    nc.tensor.matmul(ps[:], lhsT=xTo16_sb[:, w * 128:(w + 1) * 128],
                                     rhs=wemb16_sb[:], start=True, stop=True)
                    nc.scalar.copy(h0own[:, w, :], ps[:])
                # own-range h0 transposed (fp32 GRU state)
                for c0 in range(0, npc, 512):
                    cn = min(512, npc - c0)
                    ps = pp1.tile([D, 512], F32, tag="pshT")
                    nc.tensor.matmul(ps[:, 0:cn], lhsT=wemb_sb[:],
                                     rhs=xTo_sb[:, c0:c0 + cn], start=True, stop=True)
                    nc.vector.tensor_copy(h_bufs[0][:, c0:c0 + cn], ps[:, 0:cn])

            # gather round 1: h0[src] -> G (waits for h_full0 via prep deps)
            if not no_trig:
                trigger_gather(1, h_full0[0:1, 0:2],
                               prep_args=(iA, h_full0[:, :]))

            # ---------------- A2: edge MLP -> We ----------------
            with tc.tile_pool(name="pHD", bufs=1, space="PSUM") as phd, \
                 tc.tile_pool(name="pHE", bufs=1, space="PSUM") as phe, \
                 tc.tile_pool(name="pW", bufs=2, space="PSUM") as pw, \
                 tc.tile_pool(name="sA2", bufs=3) as sp2, \
                 tc.tile_pool(name="sST", bufs=1) as spst, \
                 tc.tile_pool(name="sWt", bufs=3) as spw:
                ST_sb = spst.tile([128, ntpc * 128], F8, name="ST_sb")
                nc.sync.dma_start(ST_sb[:],
                                  t_stmat.ap().rearrange("p t e -> p (t e)"))
                evac_flip = 0
                for t0 in (range(0, ntpc, 4) if not no_a2 else []):  # 4 tiles = 512 edges
                    tn = 4
                    en = tn * 128
                    # hdT via one-hot ST matmuls (dst is window-local)
                    pshd = phd.tile([D, 512], F32, tag="pshd")
                    j = 0
                    while j < tn:
                        gt = t0 + j
                        w = gt // T
                        j2 = j
                        while j2 < tn and (t0 + j2) // T == w:
                            j2 += 1
                        nc.tensor.matmul(
                            pshd[:, j * 128:j2 * 128],
                            lhsT=h0own[:, w, :],
                            rhs=ST_sb[:, gt * 128:(t0 + j2) * 128],
                            start=True, stop=True)
                        j = j2
                    # hsT for the 4 tiles via one batched PE transpose:
                    # in [128e, (4t, 32d)] -> psum [(4t, 32d), 128e]
                    Gc = sp2.tile([128, 128], F16, tag="Gc")
                    nc.vector.tensor_copy(
                        Gc[:].rearrange("p (t d) -> p t d", t=4),
                        G[:, t0:t0 + 4, 0:D])
                    pshs = phd.tile([128, 128], F16, tag="pshs")
                    nc.tensor.transpose(pshs[:], Gc[:], idm16[:])
                    hsT = sp2.tile([D, 512], F16, tag="hsT")
                    for j in range(tn):
                        nc.vector.tensor_copy(hsT[:, j * 128:(j + 1) * 128],
                                              pshs[j * D:(j + 1) * D, :])
                    dT = sp2.tile([D, 512], F16, tag="dT")
                    nc.vector.tensor_sub(dT[:, 0:en], pshd[:, 0:en], hsT[:, 0:en])
                    # he = relu(Wth dT + Wph hsT + b)
                    psh = phe.tile([D, 512], F32, tag="pshe")
                    nc.tensor.matmul(psh[:, 0:en], lhsT=wth_sb[:],
                                     rhs=dT[:, 0:en], start=True, stop=False)
                    nc.tensor.matmul(psh[:, 0:en], lhsT=wph_sb[:],
                                     rhs=hsT[:, 0:en], start=False, stop=True)
                    he = sp2.tile([D, 512], F16, tag="he")
                    nc.scalar.activation(he[:, 0:en], psh[:, 0:en], AF.Relu,
                                         bias=btp_sb[:])
                    psg = phe.tile([64, 512], F32, tag="psg")
                    nc.tensor.matmul(psg[:, 0:en], lhsT=w1_sb[:], rhs=he[:, 0:en],
                                     start=True, stop=True)
                    ga = sp2.tile([65, 512], BF16, tag="ga")
                    nc.vector.memset(ga[64:65, 0:en], 1.0)
                    nc.scalar.activation(ga[0:64, 0:en], psg[:, 0:en], AF.Relu,
                                         bias=b1c_sb[:])
                    for j in range(tn):
                        gt = t0 + j
                        s0 = j * 128
                        pw0 = pw.tile([128, 512], F32, tag="psw0")
                        pw1 = pw.tile([128, 512], F32, tag="psw1")
                        nc.tensor.matmul(pw0[:], lhsT=ga[:, s0:s0 + 128],
                                         rhs=w2_sb[:, 0:512], start=True, stop=True)
                        nc.tensor.matmul(pw1[:], lhsT=ga[:, s0:s0 + 128],
                                         rhs=w2_sb[:, 512:1024], start=True, stop=True)
                        wt = spw.tile([128, 1024], F16, tag="wt")
                        if evac_flip == 0:
                            nc.scalar.copy(wt[:, 0:512], pw0[:])
                            nc.vector.tensor_copy(wt[:, 512:1024], pw1[:])
                        else:
                            nc.vector.tensor_copy(wt[:, 0:512], pw0[:])
                            nc.scalar.copy(wt[:, 512:1024], pw1[:])
                        evac_flip ^= 1
                        nc.sync.dma_start(
                            we_dram[gt * 128:(gt + 1) * 128, :], wt[:])

            _apool_cm.__exit__(None, None, None)

            # ---------------- Phase B: message passing steps ----------------
            we_view = we_dram[:].rearrange("(t p) f -> p t f", p=128)
            with tc.tile_pool(name="sWq", bufs=2) as swq, \
                 tc.tile_pool(name="sPr", bufs=2) as spr, \
                 tc.tile_pool(name="sP16", bufs=2) as sp16, \
                 tc.tile_pool(name="sWin", bufs=2) as swin, \
                 tc.tile_pool(name="sGru", bufs=1) as sgru, \
                 tc.tile_pool(name="pA", bufs=2, space="PSUM") as ppa, \
                 tc.tile_pool(name="pT", bufs=1, space="PSUM") as ppt, \
                 tc.tile_pool(name="pG", bufs=1, space="PSUM") as ppg:
                for step in range(steps_exec):
                    h_cur = h_bufs[step % 2]
                    h_new = h_bufs[(step + 1) % 2]

                    if step > 0:
                        trigger_gather(step + 1, h_fulls[step - 1][0:1, 0:2],
                                       prep_args=(iB, h_fulls[step - 1][:, :]))

                    aT = sgru.tile([D, npc], F32, tag="aT")
                    psa = None
                    for q0 in range(0, ntpc, 4):
                        k = min(4, ntpc - q0)
                        wq = swq.tile([128, 4, 1024], F16, tag="wq")
                        nc.sync.dma_start(wq[:, 0:k, :], we_view[:, q0:q0 + k, :])
                        prod = spr.tile([128, 4, D, D], F16, tag="prod")
                        base = G[:, q0:q0 + k, 0:D]
                        in1 = _bcast_mid(base, D)
                        nc.vector.tensor_tensor(
                            prod[:, 0:k, :, :],
                            wq[:, 0:k, :].rearrange("p t (o i) -> p t o i", o=D),
                            in1, op=OP.mult)
                        p16 = sp16.tile([128, 4, D, 16], F16, tag="p16")
                        nc.vector.tensor_tensor(
                            p16[:, 0:k, :, :], prod[:, 0:k, :, 0:16],
                            prod[:, 0:k, :, 16:32], op=OP.add)
                        for j in range(k):
                            gt = q0 + j
                            w = gt // T
                            tloc = gt % T
                            if tloc == 0:
                                psa = ppa.tile([128, 512], F32, tag="psa")
                            nc.tensor.matmul(
                                psa[:], lhsT=S_sb[:, gt * 128:(gt + 1) * 128],
                                rhs=p16[:, j, :, :],
                                start=(tloc == 0), stop=(tloc == T - 1))
                            if tloc == T - 1:
                                aw = swin.tile([128, D, 16], F32, tag="aw")
                                nc.scalar.copy(
                                    aw[:], psa[:].rearrange("p (o i) -> p o i", o=D))
                                t8 = swin.tile([128, D, 8], F32, tag="t8")
                                nc.vector.tensor_tensor(t8[:], aw[:, :, 0:8],
                                                        aw[:, :, 8:16], op=OP.add)
                                t4 = swin.tile([128, D, 4], F32, tag="t4")
                                nc.vector.tensor_tensor(t4[:], t8[:, :, 0:4],
                                                        t8[:, :, 4:8], op=OP.add)
                                t2 = swin.tile([128, D, 2], F32, tag="t2")
                                nc.vector.tensor_tensor(t2[:], t4[:, :, 0:2],
                                                        t4[:, :, 2:4], op=OP.add)
                                t1 = swin.tile([128, D], F32, tag="t1")
                                nc.vector.tensor_tensor(t1[:], t2[:, :, 0],
                                                        t2[:, :, 1], op=OP.add)
                                pst = ppt.tile([D, 128], F32, tag="pst")
                                nc.tensor.transpose(pst[:], t1[:], idm[:])
                                nc.vector.tensor_copy(
                                    aT[:, w * 128:(w + 1) * 128], pst[:])

                    # ---- GRU (transposed layout) ----
                    for c0 in range(0, npc, 512):
                        cn = min(512, npc - c0)
                        cs = slice(c0, c0 + cn)
                        pgi = ppg.tile([3 * D, 512], F32, tag="pgi")
                        nc.tensor.matmul(pgi[:, 0:cn], lhsT=wih_sb[:],
                                         rhs=aT[:, cs], start=True, stop=True)
                        pgh = ppg.tile([3 * D, 512], F32, tag="pgh")
                        nc.tensor.matmul(pgh[:, 0:cn], lhsT=whh_sb[:],
                                         rhs=h_cur[:, cs], start=True, stop=True)
                        gh_sb = sgru.tile([3 * D, 512], F32, tag="gh_sb")
                        nc.scalar.copy(gh_sb[:, 0:cn], pgh[:, 0:cn])
                        tr = sgru.tile([D, 512], F32, tag="tr")
                        nc.vector.tensor_add(tr[:, 0:cn], pgi[0:D, 0:cn],
                                             gh_sb[0:D, 0:cn])
                        r = sgru.tile([D, 512], F32, tag="r")
                        nc.scalar.activation(r[:, 0:cn], tr[:, 0:cn], AF.Sigmoid,
                                             bias=br_sb[:])
                        tz = sgru.tile([D, 512], F32, tag="tz")
                        nc.vector.tensor_add(tz[:, 0:cn], pgi[D:2 * D, 0:cn],
                                             gh_sb[D:2 * D, 0:cn])
                        z = sgru.tile([D, 512], F32, tag="z")
                        nc.scalar.activation(z[:, 0:cn], tz[:, 0:cn], AF.Sigmoid,
                                             bias=bz_sb[:])
                        hnb = sgru.tile([D, 512], F32, tag="hnb")
                        nc.vector.tensor_scalar_add(hnb[:, 0:cn],
                                                    gh_sb[2 * D:3 * D, 0:cn],
                                                    bhn_sb[:])
                        rhn = sgru.tile([D, 512], F32, tag="rhn")
                        nc.vector.tensor_mul(rhn[:, 0:cn], r[:, 0:cn], hnb[:, 0:cn])
                        tn_ = sgru.tile([D, 512], F32, tag="tn_")
                        nc.vector.tensor_add(tn_[:, 0:cn], rhn[:, 0:cn],
                                             pgi[2 * D:3 * D, 0:cn])
                        ngate = sgru.tile([D, 512], F32, tag="ngate")
                        nc.scalar.activation(ngate[:, 0:cn], tn_[:, 0:cn], AF.Tanh,
                                             bias=bin_sb[:])
                        hmn = sgru.tile([D, 512], F32, tag="hmn")
                        nc.vector.tensor_sub(hmn[:, 0:cn], h_cur[:, cs],
                                             ngate[:, 0:cn])
                        zh = sgru.tile([D, 512], F32, tag="zh")
                        nc.vector.tensor_mul(zh[:, 0:cn], z[:, 0:cn], hmn[:, 0:cn])
                        nc.vector.tensor_add(h_new[:, cs], ngate[:, 0:cn],
                                             zh[:, 0:cn])

                    # ---- write h out; AllGather (except after last step) ----
                    if step < steps_exec - 1:
                        hst = sgru.tile([128, wpc, 128], F16, tag="hst")
                        if step == 0:
                            nc.vector.memset(hst[:], 0.0)
                        for w in range(wpc):
                            ps2 = ppt.tile([128, D], F32, tag="ps2")
                            nc.tensor.transpose(ps2[:],
                                                h_new[:, w * 128:(w + 1) * 128],
                                                idm[0:D, 0:D])
                            nc.scalar.copy(hst[:, w, 0:D], ps2[:])
                        nc.sync.dma_start(
                            cc_ins[step][:].rearrange("(p w) f -> p (w f)", p=128),
                            hst[:])
                        nc.gpsimd.collective_compute(
                            "AllGather", OP.bypass,
                            replica_groups=[list(range(ncores))],
                            ins=[cc_ins[step][:].opt()],
                            outs=[h_fulls[step][:].opt()])
                    else:
                        ost = sgru.tile([128, wpc, D], F32, tag="ost")
                        for w in range(wpc):
                            ps2 = ppt.tile([128, D], F32, tag="ps2")
                            nc.tensor.transpose(ps2[:],
                                                h_new[:, w * 128:(w + 1) * 128],
                                                idm[0:D, 0:D])
                            nc.scalar.copy(ost[:, w, :], ps2[:])
                        nc.sync.dma_start(
                            t_out.ap().rearrange("(w p) d -> p w d", p=128), ost[:])

        if steps_exec == 0:
            with tc.tile_pool(name="sZ", bufs=1) as sz:
                zst = sz.tile([128, wpc, D], F32, name="zst")
                nc.vector.memset(zst[:], 0.0)
                nc.sync.dma_start(
                    t_out.ap().rearrange("(w p) d -> p w d", p=128), zst[:])
    nc.compile()
    return nc


# --------------------------------------------------------------------------- #
# entry point
# --------------------------------------------------------------------------- #

def run(inputs, n_nodes=N_NODES, npc=NPC, **spmd_kwargs):
    global LAST_RESULT, LAST_META
    shared, per_core, meta = host_prep(**inputs, n_nodes=n_nodes, npc=npc)
    LAST_META = meta
    nc = build_nc(meta)
    in_maps = [dict(shared, **pc) for pc in per_core]
    res = bass_utils.run_bass_kernel_spmd(
        nc, in_maps, core_ids=list(range(meta["ncores"])), **spmd_kwargs)
    LAST_RESULT = res
    out = np.concatenate([res.results[c]["out_h"] for c in range(meta["ncores"])], 0)
    return np.ascontiguousarray(out[:n_nodes]).astype(np.float32)


def kernel(**inputs):
    return run(inputs)


# revision 28
# speedup vs baseline: 1.5303x; 1.0041x over previous
"""Trainium2 Bass kernel for nn_GNN_GRU_83519934038653 (GatedGraphConv-style GNN).

Strategy (8 NeuronCores, SPMD, one NEFF):
  - Host: sort edges by dst, shard by dst node-range (1280 nodes/core), build
    int16 gather index tables + per-tile one-hot scatter matrices S (and their
    per-tile transposes ST for the dst-side select), permute weights.
  - h tables in DRAM use a host-chosen row permutation so every SBUF->DRAM h
    write is one contiguous descriptor per partition; the gather index tables
    absorb the permutation.
  - Gathers run as prepare_only descriptor generation on SWDGE queues 1-3
    (hidden under compute) + trigger_dma when the source table is ready;
    pre-staged descriptors drain at ~250 GB/s vs ~40 GB/s gen-paced.
  - A2: one gather of h0[src] shared with step 1; h0[dst] needs no gather at
    all (dst is window-local: one-hot ST matmuls against h0 windows).
  - Phase B x3: wq = We chunk DMA; DVE multiply (h broadcast over o);
    one tree-add level; PE scatter-matmul with one-hot S; per-window tree;
    transposed GRU; AllGather h (except last step).
"""

import os
import sys

for _p in ("/opt/trn_rl_repo", "/root/.axon_site/_ro/trn_rl_repo"):
    if os.path.isdir(_p) and _p not in sys.path:
        sys.path.insert(0, _p)

import numpy as np

import concourse.bass as bass
import concourse.bacc as bacc
import concourse.mybir as mybir
import concourse.tile as tile
import concourse.bass_utils as bass_utils

F32 = mybir.dt.float32
F16 = mybir.dt.float16
F8 = mybir.dt.float8e4
BF16 = mybir.dt.bfloat16
I16 = mybir.dt.int16
AF = mybir.ActivationFunctionType
OP = mybir.AluOpType

N_NODES = 10000
N_EDGES = 160000
D = 32
IN_DIM = 2
STEPS = 3
CORES = 8
NPC = 1280   # padded nodes per core; 8*1280 = 10240
WIN = 128    # scatter window (nodes per PSUM accumulation window)
NCH = 4      # gather chunks (SWDGE queues 0..NCH-1)

S_NP = mybir.dt.np(F8)
import ml_dtypes
_BF16_NP = ml_dtypes.bfloat16

LAST_RESULT = None
LAST_META = None


# --------------------------------------------------------------------------- #
# host-side preparation
# --------------------------------------------------------------------------- #

def _wrap_idx(idx, epc):
    w = idx.reshape(epc // 16, 16).T.astype(np.int16)
    return np.tile(w, (8, 1)).copy()


def _rowmap_A(n):
    """h_full0 row of node n: h0st[p, w] holds node w*128+p, stored p-major."""
    return (n % 128) * 80 + n // 128


def _rowmap_B(n):
    """h_full(step) row of node n after AllGather of p-major cc_in shards."""
    c = n // NPC
    loc = n % NPC
    return c * NPC + (loc % 128) * (NPC // 128) + loc // 128


def host_prep(x, src, dst, W_emb, b_emb, W_theta, b_theta, W_phi, b_phi,
              W1, b1, W2, b2, W_ih, b_ih, W_hh, b_hh,
              n_nodes=N_NODES, npc=NPC, ncores=CORES):
    n_pad = npc * ncores
    wpc = npc // WIN
    nwin = ncores * wpc

    src = np.asarray(src).astype(np.int64)
    dst = np.asarray(dst).astype(np.int64)
    order = np.argsort(dst, kind="stable")
    src_s = src[order]
    dst_s = dst[order]

    win_of_edge = dst_s // WIN
    counts = np.bincount(win_of_edge, minlength=nwin)
    T = max(1, int(np.ceil(counts.max() / 128)))
    ntpc = wpc * T
    epc = ntpc * 128
    assert epc % (16 * NCH) == 0 and ntpc % NCH == 0

    win_start = np.concatenate([[0], np.cumsum(counts)])

    per_core = []
    spread = (np.arange(epc, dtype=np.int64) * 127) % n_pad
    for c in range(ncores):
        src_idx = spread.copy()
        S = np.zeros((128, ntpc, 128), np.float32)
        for w in range(wpc):
            g = c * wpc + w
            e0, e1 = int(win_start[g]), int(win_start[g + 1])
            k = e1 - e0
            if k == 0:
                continue
            base = w * T * 128
            j = np.arange(k)
            src_idx[base + j] = src_s[e0:e1]
            dloc = dst_s[e0:e1] - (c * npc + w * WIN)
            assert (dloc >= 0).all() and (dloc < WIN).all()
            S[j % 128, w * T + j // 128, dloc] = 1.0
        ST = np.ascontiguousarray(S.transpose(2, 1, 0))  # [dloc, tile, e]
        per_core.append({
            "srcA_w": _wrap_idx(_rowmap_A(src_idx), epc),
            "srcB_w": _wrap_idx(_rowmap_B(src_idx), epc),
            "s_mat": S.astype(S_NP),
            "st_mat": ST.astype(S_NP),
            "xT_own": None,
        })

    f32 = np.float32
    f16 = np.float16
    x = np.asarray(x, f32)
    x_pad = np.zeros((n_pad, IN_DIM), f32)
    x_pad[:n_nodes] = x
    xT_aug = np.concatenate([x_pad.T, np.ones((1, n_pad), f32)], 0)  # [3, n_pad]
    for c in range(ncores):
        per_core[c]["xT_own"] = np.ascontiguousarray(
            xT_aug[:, c * npc:(c + 1) * npc])
        per_core[c]["xTo16"] = per_core[c]["xT_own"].astype(f16)

    W_emb_aug = np.concatenate([np.asarray(W_emb, f32),
                                np.asarray(b_emb, f32)[None, :]], 0)  # [3, 32]

    W2p = np.asarray(W2, f32).reshape(64, D, D).transpose(0, 2, 1).reshape(64, D * D)
    b2p = np.asarray(b2, f32).reshape(D, D).T.reshape(D * D)
    W2pa = np.concatenate([W2p, b2p[None, :]], 0).astype(f16)  # [65, 1024]

    shared = {
        "xT_aug": xT_aug,
        "xT16": xT_aug.astype(f16),
        "w_emb": W_emb_aug,
        "w_emb16": W_emb_aug.astype(f16),
        "w_theta": np.asarray(W_theta, f32).astype(f16),
        "w_phi": np.asarray(W_phi, f32).astype(f16),
        "w1": np.asarray(W1, f32).astype(f16),
        "w2pa": W2pa.astype(np.float32).astype(_BF16_NP),
        "w_ih": np.asarray(W_ih, f32),
        "w_hh": np.asarray(W_hh, f32),
        "b_tp": (np.asarray(b_theta, f32) + np.asarray(b_phi, f32))[:, None],
        "b1c": np.asarray(b1, f32)[:, None],
        "b_r": (np.asarray(b_ih, f32)[0:D] + np.asarray(b_hh, f32)[0:D])[:, None],
        "b_z": (np.asarray(b_ih, f32)[D:2 * D] + np.asarray(b_hh, f32)[D:2 * D])[:, None],
        "b_in": np.asarray(b_ih, f32)[2 * D:3 * D][:, None],
        "b_hn": np.asarray(b_hh, f32)[2 * D:3 * D][:, None],
        "idm": np.eye(128, dtype=f32),
        "idm16": np.eye(128).astype(f16),
    }
    meta = dict(T=T, npc=npc, ncores=ncores, n_pad=n_pad, wpc=wpc,
                ntpc=ntpc, epc=epc, steps=STEPS)
    return shared, per_core, meta


# --------------------------------------------------------------------------- #
# device kernel builder
# --------------------------------------------------------------------------- #

def _bcast_mid(ap_base, count):
    aps = [list(p) for p in ap_base.ap]
    new = aps[:-1] + [[0, count]] + [aps[-1]]
    return bass.AP(ap_base.tensor, ap_base.offset, new)


def build_nc(meta):
    T = meta["T"]; npc = meta["npc"]; ncores = meta["ncores"]
    n_pad = meta["n_pad"]; wpc = meta["wpc"]; ntpc = meta["ntpc"]
    epc = meta["epc"]; steps = meta["steps"]
    tpch = ntpc // NCH           # tiles per gather chunk
    epch = tpch * 128            # edges per gather chunk
    assert epch % 16 == 0
    steps_exec = int(os.environ.get("K_STEPS", steps))
    no_a2 = bool(int(os.environ.get("K_NOA2", "0")))
    no_trig = bool(int(os.environ.get("K_NOTRIG", "0")))
    no_sig = bool(int(os.environ.get("K_NOSIG", "0")))
    no_touch = bool(int(os.environ.get("K_NOTOUCH", "0")))

    nc = bacc.Bacc("TRN2", target_bir_lowering=False, debug=False,
                   enable_asserts=False, num_devices=ncores,
                   num_swdge_queues=4)
    global _DBG_NC
    _DBG_NC = nc

    # ---- I/O tensors ----
    t_xT = nc.dram_tensor("xT_aug", [IN_DIM + 1, n_pad], F32, kind="ExternalInput")
    t_xT16 = nc.dram_tensor("xT16", [IN_DIM + 1, n_pad], F16, kind="ExternalInput")
    t_xTo = nc.dram_tensor("xT_own", [IN_DIM + 1, npc], F32, kind="ExternalInput")
    t_xTo16 = nc.dram_tensor("xTo16", [IN_DIM + 1, npc], F16, kind="ExternalInput")
    t_wemb = nc.dram_tensor("w_emb", [IN_DIM + 1, D], F32, kind="ExternalInput")
    t_wemb16 = nc.dram_tensor("w_emb16", [IN_DIM + 1, D], F16, kind="ExternalInput")
    t_wth = nc.dram_tensor("w_theta", [D, D], F16, kind="ExternalInput")
    t_wph = nc.dram_tensor("w_phi", [D, D], F16, kind="ExternalInput")
    t_w1 = nc.dram_tensor("w1", [D, 64], F16, kind="ExternalInput")
    t_w2 = nc.dram_tensor("w2pa", [65, 1024], BF16, kind="ExternalInput")
    t_wih = nc.dram_tensor("w_ih", [D, 3 * D], F32, kind="ExternalInput")
    t_whh = nc.dram_tensor("w_hh", [D, 3 * D], F32, kind="ExternalInput")
    t_btp = nc.dram_tensor("b_tp", [D, 1], F32, kind="ExternalInput")
    t_b1c = nc.dram_tensor("b1c", [64, 1], F32, kind="ExternalInput")
    t_br = nc.dram_tensor("b_r", [D, 1], F32, kind="ExternalInput")
    t_bz = nc.dram_tensor("b_z", [D, 1], F32, kind="ExternalInput")
    t_bin = nc.dram_tensor("b_in", [D, 1], F32, kind="ExternalInput")
    t_bhn = nc.dram_tensor("b_hn", [D, 1], F32, kind="ExternalInput")
    t_idm = nc.dram_tensor("idm", [128, 128], F32, kind="ExternalInput")
    t_idm16 = nc.dram_tensor("idm16", [128, 128], F16, kind="ExternalInput")
    t_srcA = nc.dram_tensor("srcA_w", [128, epc // 16], I16, kind="ExternalInput")
    t_srcB = nc.dram_tensor("srcB_w", [128, epc // 16], I16, kind="ExternalInput")
    t_smat = nc.dram_tensor("s_mat", [128, ntpc, 128], F8, kind="ExternalInput")
    t_stmat = nc.dram_tensor("st_mat", [128, ntpc, 128], F8, kind="ExternalInput")
    t_out = nc.dram_tensor("out_h", [npc, D], F32, kind="ExternalOutput")

    sem_q = [nc.alloc_semaphore(f"gsem_q{q}") for q in range(NCH)]
    psem = nc.alloc_semaphore("gprep_sem")

    with tile.TileContext(nc) as tc:
        with tc.tile_pool(name="dram", bufs=1, space="DRAM") as dpool, \
             tc.tile_pool(name="const", bufs=1) as cpool:
            we_dram = dpool.tile([epc, 1024], F16, name="we_dram")
            h_full0 = dpool.tile([n_pad, 128], F16, name="h_full0")
            h_fulls = [dpool.tile([n_pad, 128], F16, addr_space="Shared",
                                  name=f"h_full{s}") for s in (1, 2)]
            cc_ins = [dpool.tile([npc, 128], F16, name=f"cc_in{s}")
                      for s in range(steps - 1)]

            # resident constants
            idm = cpool.tile([128, 128], F32, name="idm")
            nc.sync.dma_start(idm[:], t_idm.ap())
            idm16 = cpool.tile([128, 128], F16, name="idm16")
            nc.sync.dma_start(idm16[:], t_idm16.ap())
            touch = cpool.tile([1, 2], F16, name="touch")
            tsem = nc.alloc_semaphore("touch_sem")
            S_sb = cpool.tile([128, ntpc * 128], F8, name="S_sb")
            nc.sync.dma_start(S_sb[:], t_smat.ap().rearrange("p t e -> p (t e)"))
            iA = cpool.tile([128, epc // 16], I16, name="iA")
            nc.sync.dma_start(iA[:], t_srcA.ap())
            iB = cpool.tile([128, epc // 16], I16, name="iB")
            nc.sync.dma_start(iB[:], t_srcB.ap())

            def load_const(t, shape, dtype, name):
                s = cpool.tile(shape, dtype, name=name)
                nc.sync.dma_start(s[:], t.ap())
                return s

            xTo_sb = load_const(t_xTo, [IN_DIM + 1, npc], F32, "xTo_sb")
            wemb_sb = load_const(t_wemb, [IN_DIM + 1, D], F32, "wemb_sb")
            wemb16_sb = load_const(t_wemb16, [IN_DIM + 1, D], F16, "wemb16_sb")
            wth_sb = load_const(t_wth, [D, D], F16, "wth_sb")
            wph_sb = load_const(t_wph, [D, D], F16, "wph_sb")
            w1_sb = load_const(t_w1, [D, 64], F16, "w1_sb")
            w2_sb = load_const(t_w2, [65, 1024], BF16, "w2_sb")
            wih_sb = load_const(t_wih, [D, 3 * D], F32, "wih_sb")
            whh_sb = load_const(t_whh, [D, 3 * D], F32, "whh_sb")
            btp_sb = load_const(t_btp, [D, 1], F32, "btp_sb")
            b1c_sb = load_const(t_b1c, [64, 1], F32, "b1c_sb")
            br_sb = load_const(t_br, [D, 1], F32, "br_sb")
            bz_sb = load_const(t_bz, [D, 1], F32, "bz_sb")
            bin_sb = load_const(t_bin, [D, 1], F32, "bin_sb")
            bhn_sb = load_const(t_bhn, [D, 1], F32, "bhn_sb")

            # GRU state (transposed layout), ping-pong across steps
            h_bufs = [cpool.tile([D, npc], F32, name=f"hT{i}") for i in range(2)]
            # gathered h[src] (shared by A2 and each step's DVE stage)
            G = cpool.tile([128, ntpc, 128], F16, name="G")

            def prep_gather(idx_sb, h_src_ap):
                for c in range(NCH):
                    i = nc.gpsimd.dma_gather(
                        G[:, c * tpch:(c + 1) * tpch, :], h_src_ap,
                        idx_sb[:, c * (epch // 16):(c + 1) * (epch // 16)],
                        epch, epch, 128,
                        transpose=False, single_packet=False,
                        prepare_only=True, sem=sem_q[c], queue_num=c)
                    i.then_inc(psem, 1)

            def trigger_gather(round_no, src_ap, prep_args=None):
                # trigger + completion waits in one critical; preps either
                # inline (round 1) or emitted earlier (bare) to hide desc-gen
                with tc.tile_critical():
                    if prep_args is not None:
                        prep_gather(*prep_args)
                    nc.gpsimd.wait_ge(psem, NCH * round_no)
                    for c in range(NCH):
                        nc.gpsimd.trigger_dma(count=1, queue_num=c)
                    for c in range(NCH):
                        nc.gpsimd.wait_ge(sem_q[c], 16 * round_no)

            late_prep = int(os.environ.get("K_LATEPREP", "0"))


            # ---------------- A1 + A2 scoped pool ----------------
            _apool_cm = tc.tile_pool(name="aph", bufs=1)
            apool = _apool_cm.__enter__()
            h0st = apool.tile([128, 80, 128], F16, name="h0st")
            xT16 = apool.tile([IN_DIM + 1, n_pad], F16, name="xT16")
            nc.sync.dma_start(xT16[:], t_xT16.ap())
            # ---------------- A1: h0 ----------------
            with tc.tile_pool(name="pA1", bufs=2, space="PSUM") as pp1:
                nc.vector.memset(h0st[:], 0.0)
                for ch in range(80):
                    ps = pp1.tile([128, D], F32, tag="psh0")
                    nc.tensor.matmul(ps[:], lhsT=xT16[:, ch * 128:(ch + 1) * 128],
                                     rhs=wemb16_sb[:], start=True, stop=True)
                    nc.scalar.copy(h0st[:, ch, 0:D], ps[:])
                # one contiguous-per-partition descriptor per partition
                if not bool(int(os.environ.get("K_NOSTORE", "0"))):
                    nc.sync.dma_start(
                        h_full0[:].rearrange("(p t) f -> p (t f)", p=128), h0st[:])
                # own-window node-major h0 (dst-side stationaries for A2)
                xTo16_sb = apool.tile([IN_DIM + 1, npc], F16, name="xTo16_sb")
                nc.sync.dma_start(xTo16_sb[:], t_xTo16.ap())
                h0own = apool.tile([128, wpc, D], F16, name="h0own")
                for w in range(wpc):
                    ps = pp1.tile([128, D], F32, tag="psh0")
                    nc.tensor.matmul(ps[:], lhsT=xTo16_sb[:, w * 128:(w + 1) * 128],
                                     rhs=wemb16_sb[:], start=True, stop=True)
                    nc.scalar.copy(h0own[:, w, :], ps[:])
                # own-range h0 transposed (fp32 GRU state)
                for c0 in range(0, npc, 512):
                    cn = min(512, npc - c0)
                    ps = pp1.tile([D, 512], F32, tag="pshT")
                    nc.tensor.matmul(ps[:, 0:cn], lhsT=wemb_sb[:],
                                     rhs=xTo_sb[:, c0:c0 + cn], start=True, stop=True)
                    nc.vector.tensor_copy(h_bufs[0][:, c0:c0 + cn], ps[:, 0:cn])

            # gather round 1: h0[src] -> G (waits for h_full0 via prep deps)
            if not no_trig:
                trigger_gather(1, h_full0[0:1, 0:2],
                               prep_args=(iA, h_full0[:, :]))

            # ---------------- A2: edge MLP -> We ----------------
            with tc.tile_pool(name="pHD", bufs=1, space="PSUM") as phd, \
                 tc.tile_pool(name="pHE", bufs=1, space="PSUM") as phe, \
                 tc.tile_pool(name="pW", bufs=2, space="PSUM") as pw, \
                 tc.tile_pool(name="sA2", bufs=3) as sp2, \
                 tc.tile_pool(name="sST", bufs=1) as spst, \
                 tc.tile_pool(name="sWt", bufs=3) as spw:
                ST_sb = spst.tile([128, ntpc * 128], F8, name="ST_sb")
                nc.sync.dma_start(ST_sb[:],
                                  t_stmat.ap().rearrange("p t e -> p (t e)"))
                evac_flip = 0
                for t0 in (range(0, ntpc, 4) if not no_a2 else []):  # 4 tiles = 512 edges
                    tn = 4
                    en = tn * 128
                    # hdT via one-hot ST matmuls (dst is window-local)
                    pshd = phd.tile([D, 512], F32, tag="pshd")
                    j = 0
                    while j < tn:
                        gt = t0 + j
                        w = gt // T
                        j2 = j
                        while j2 < tn and (t0 + j2) // T == w:
                            j2 += 1
                        nc.tensor.matmul(
                            pshd[:, j * 128:j2 * 128],
                            lhsT=h0own[:, w, :],
                            rhs=ST_sb[:, gt * 128:(t0 + j2) * 128],
                            start=True, stop=True)
                        j = j2
                    # hsT for the 4 tiles via one batched PE transpose:
                    # in [128e, (4t, 32d)] -> psum [(4t, 32d), 128e]
                    Gc = sp2.tile([128, 128], F16, tag="Gc")
                    nc.vector.tensor_copy(
                        Gc[:].rearrange("p (t d) -> p t d", t=4),
                        G[:, t0:t0 + 4, 0:D])
                    pshs = phd.tile([128, 128], F16, tag="pshs")
                    nc.tensor.transpose(pshs[:], Gc[:], idm16[:])
                    hsT = sp2.tile([D, 512], F16, tag="hsT")
                    for j in range(tn):
                        nc.vector.tensor_copy(hsT[:, j * 128:(j + 1) * 128],
                                              pshs[j * D:(j + 1) * D, :])
                    dT = sp2.tile([D, 512], F16, tag="dT")
                    nc.vector.tensor_sub(dT[:, 0:en], pshd[:, 0:en], hsT[:, 0:en])
                    # he = relu(Wth dT + Wph hsT + b)
                    psh = phe.tile([D, 512], F32, tag="pshe")
                    nc.tensor.matmul(psh[:, 0:en], lhsT=wth_sb[:],
                                     rhs=dT[:, 0:en], start=True, stop=False)
                    nc.tensor.matmul(psh[:, 0:en], lhsT=wph_sb[:],
                                     rhs=hsT[:, 0:en], start=False, stop=True)
                    he = sp2.tile([D, 512], F16, tag="he")
                    nc.scalar.activation(he[:, 0:en], psh[:, 0:en], AF.Relu,
                                         bias=btp_sb[:])
                    psg = phe.tile([64, 512], F32, tag="psg")
                    nc.tensor.matmul(psg[:, 0:en], lhsT=w1_sb[:], rhs=he[:, 0:en],
                                     start=True, stop=True)
                    ga = sp2.tile([65, 512], BF16, tag="ga")
                    nc.vector.memset(ga[64:65, 0:en], 1.0)
                    nc.scalar.activation(ga[0:64, 0:en], psg[:, 0:en], AF.Relu,
                                         bias=b1c_sb[:])
                    for j in range(tn):
                        gt = t0 + j
                        s0 = j * 128
                        pw0 = pw.tile([128, 512], F32, tag="psw0")
                        pw1 = pw.tile([128, 512], F32, tag="psw1")
                        nc.tensor.matmul(pw0[:], lhsT=ga[:, s0:s0 + 128],
                                         rhs=w2_sb[:, 0:512], start=True, stop=True)
                        nc.tensor.matmul(pw1[:], lhsT=ga[:, s0:s0 + 128],
                                         rhs=w2_sb[:, 512:1024], start=True, stop=True)
                        wt = spw.tile([128, 1024], F16, tag="wt")
                        if evac_flip == 0:
                            nc.scalar.copy(wt[:, 0:512], pw0[:])
                            nc.vector.tensor_copy(wt[:, 512:1024], pw1[:])
                        else:
                            nc.vector.tensor_copy(wt[:, 0:512], pw0[:])
                            nc.scalar.copy(wt[:, 512:1024], pw1[:])
                        evac_flip ^= 1
                        nc.sync.dma_start(
                            we_dram[gt * 128:(gt + 1) * 128, :], wt[:])

            _apool_cm.__exit__(None, None, None)

            # ---------------- Phase B: message passing steps ----------------
            we_view = we_dram[:].rearrange("(t p) f -> p t f", p=128)
            with tc.tile_pool(name="sWq", bufs=2) as swq, \
                 tc.tile_pool(name="sPr", bufs=2) as spr, \
                 tc.tile_pool(name="sP16", bufs=2) as sp16, \
                 tc.tile_pool(name="sWin", bufs=2) as swin, \
                 tc.tile_pool(name="sGru", bufs=1) as sgru, \
                 tc.tile_pool(name="pA", bufs=2, space="PSUM") as ppa, \
                 tc.tile_pool(name="pT", bufs=1, space="PSUM") as ppt, \
                 tc.tile_pool(name="pG", bufs=1, space="PSUM") as ppg:
                for step in range(steps_exec):
                    h_cur = h_bufs[step % 2]
                    h_new = h_bufs[(step + 1) % 2]

                    if step > 0:
                        trigger_gather(step + 1, h_fulls[step - 1][0:1, 0:2],
                                       prep_args=(iB, h_fulls[step - 1][:, :]))

                    aT = sgru.tile([D, npc], F32, tag="aT")
                    psa = None
                    for q0 in range(0, ntpc, 4):
                        k = min(4, ntpc - q0)
                        wq = swq.tile([128, 4, 1024], F16, tag="wq")
                        nc.sync.dma_start(wq[:, 0:k, :], we_view[:, q0:q0 + k, :])
                        prod = spr.tile([128, 4, D, D], F16, tag="prod")
                        base = G[:, q0:q0 + k, 0:D]
                        in1 = _bcast_mid(base, D)
                        nc.vector.tensor_tensor(
                            prod[:, 0:k, :, :],
                            wq[:, 0:k, :].rearrange("p t (o i) -> p t o i", o=D),
                            in1, op=OP.mult)
                        p16 = sp16.tile([128, 4, D, 16], F16, tag="p16")
                        nc.vector.tensor_tensor(
                            p16[:, 0:k, :, :], prod[:, 0:k, :, 0:16],
                            prod[:, 0:k, :, 16:32], op=OP.add)
                        for j in range(k):
                            gt = q0 + j
                            w = gt // T
                            tloc = gt % T
                            if tloc == 0:
                                psa = ppa.tile([128, 512], F32, tag="psa")
                            nc.tensor.matmul(
                                psa[:], lhsT=S_sb[:, gt * 128:(gt + 1) * 128],
                                rhs=p16[:, j, :, :],
                                start=(tloc == 0), stop=(tloc == T - 1))
                            if tloc == T - 1:
                                aw = swin.tile([128, D, 16], F32, tag="aw")
                                nc.scalar.copy(
                                    aw[:], psa[:].rearrange("p (o i) -> p o i", o=D))
                                t8 = swin.tile([128, D, 8], F32, tag="t8")
                                nc.vector.tensor_tensor(t8[:], aw[:, :, 0:8],
                                                        aw[:, :, 8:16], op=OP.add)
                                t4 = swin.tile([128, D, 4], F32, tag="t4")
                                nc.vector.tensor_tensor(t4[:], t8[:, :, 0:4],
                                                        t8[:, :, 4:8], op=OP.add)
                                t2 = swin.tile([128, D, 2], F32, tag="t2")
                                nc.vector.tensor_tensor(t2[:], t4[:, :, 0:2],
                                                        t4[:, :, 2:4], op=OP.add)
                                t1 = swin.tile([128, D], F32, tag="t1")
                                nc.vector.tensor_tensor(t1[:], t2[:, :, 0],
                                                        t2[:, :, 1], op=OP.add)
                                pst = ppt.tile([D, 128], F32, tag="pst")
                                nc.tensor.transpose(pst[:], t1[:], idm[:])
                                nc.vector.tensor_copy(
                                    aT[:, w * 128:(w + 1) * 128], pst[:])

                    # ---- GRU (transposed layout) ----
                    for c0 in range(0, npc, 512):
                        cn = min(512, npc - c0)
                        cs = slice(c0, c0 + cn)
                        pgi = ppg.tile([3 * D, 512], F32, tag="pgi")
                        nc.tensor.matmul(pgi[:, 0:cn], lhsT=wih_sb[:],
                                         rhs=aT[:, cs], start=True, stop=True)
                        pgh = ppg.tile([3 * D, 512], F32, tag="pgh")
                        nc.tensor.matmul(pgh[:, 0:cn], lhsT=whh_sb[:],
                                         rhs=h_cur[:, cs], start=True, stop=True)
                        gh_sb = sgru.tile([3 * D, 512], F32, tag="gh_sb")
                        nc.scalar.copy(gh_sb[:, 0:cn], pgh[:, 0:cn])
                        tr = sgru.tile([D, 512], F32, tag="tr")
                        nc.vector.tensor_add(tr[:, 0:cn], pgi[0:D, 0:cn],
                                             gh_sb[0:D, 0:cn])
                        r = sgru.tile([D, 512], F32, tag="r")
                        nc.scalar.activation(r[:, 0:cn], tr[:, 0:cn], AF.Sigmoid,
                                             bias=br_sb[:])
                        tz = sgru.tile([D, 512], F32, tag="tz")
                        nc.vector.tensor_add(tz[:, 0:cn], pgi[D:2 * D, 0:cn],
                                             gh_sb[D:2 * D, 0:cn])
                        z = sgru.tile([D, 512], F32, tag="z")
                        nc.scalar.activation(z[:, 0:cn], tz[:, 0:cn], AF.Sigmoid,
                                             bias=bz_sb[:])
                        hnb = sgru.tile([D, 512], F32, tag="hnb")
                        nc.vector.tensor_scalar_add(hnb[:, 0:cn],
                                                    gh_sb[2 * D:3 * D, 0:cn],
                                                    bhn_sb[:])
                        rhn = sgru.tile([D, 512], F32, tag="rhn")
                        nc.vector.tensor_mul(rhn[:, 0:cn], r[:, 0:cn], hnb[:, 0:cn])
                        tn_ = sgru.tile([D, 512], F32, tag="tn_")
                        nc.vector.tensor_add(tn_[:, 0:cn], rhn[:, 0:cn],
                                             pgi[2 * D:3 * D, 0:cn])
                        ngate = sgru.tile([D, 512], F32, tag="ngate")
                        nc.scalar.activation(ngate[:, 0:cn], tn_[:, 0:cn], AF.Tanh,
                                             bias=bin_sb[:])
                        hmn = sgru.tile([D, 512], F32, tag="hmn")
                        nc.vector.tensor_sub(hmn[:, 0:cn], h_cur[:, cs],
                                             ngate[:, 0:cn])
                        zh = sgru.tile([D, 512], F32, tag="zh")
                        nc.vector.tensor_mul(zh[:, 0:cn], z[:, 0:cn], hmn[:, 0:cn])
                        nc.vector.tensor_add(h_new[:, cs], ngate[:, 0:cn],
                                             zh[:, 0:cn])

                    # ---- write h out; AllGather (except after last step) ----
                    if step < steps_exec - 1:
                        hst = sgru.tile([128, wpc, 128], F16, tag="hst")
                        if step == 0:
                            nc.vector.memset(hst[:], 0.0)
                        for w in range(wpc):
                            ps2 = ppt.tile([128, D], F32, tag="ps2")
                            nc.tensor.transpose(ps2[:],
                                                h_new[:, w * 128:(w + 1) * 128],
                                                idm[0:D, 0:D])
                            nc.scalar.copy(hst[:, w, 0:D], ps2[:])
                        nc.sync.dma_start(
                            cc_ins[step][:].rearrange("(p w) f -> p (w f)", p=128),
                            hst[:])
                        nc.gpsimd.collective_compute(
                            "AllGather", OP.bypass,
                            replica_groups=[list(range(ncores))],
                            ins=[cc_ins[step][:].opt()],
                            outs=[h_fulls[step][:].opt()])
                    else:
                        ost = sgru.tile([128, wpc, D], F32, tag="ost")
                        for w in range(wpc):
                            ps2 = ppt.tile([128, D], F32, tag="ps2")
                            nc.tensor.transpose(ps2[:],
                                                h_new[:, w * 128:(w + 1) * 128],
                                                idm[0:D, 0:D])
                            nc.scalar.copy(ost[:, w, :], ps2[:])
                        nc.sync.dma_start(
                            t_out.ap().rearrange("(w p) d -> p w d", p=128), ost[:])

        if steps_exec == 0:
            with tc.tile_pool(name="sZ", bufs=1) as sz:
                zst = sz.tile([128, wpc, D], F32, name="zst")
                nc.vector.memset(zst[:], 0.0)
                nc.sync.dma_start(
                    t_out.ap().rearrange("(w p) d -> p w d", p=128), zst[:])
    nc.compile()
    return nc


# --------------------------------------------------------------------------- #
# entry point
# --------------------------------------------------------------------------- #

def run(inputs, n_nodes=N_NODES, npc=NPC, **spmd_kwargs):
    global LAST_RESULT, LAST_META
    shared, per_core, meta = host_prep(**inputs, n_nodes=n_nodes, npc=npc)
    LAST_META = meta
    nc = build_nc(meta)
    in_maps = [dict(shared, **pc) for pc in per_core]
    res = bass_utils.run_bass_kernel_spmd(
        nc, in_maps, core_ids=list(range(meta["ncores"])), **spmd_kwargs)
    LAST_RESULT = res
    out = np.concatenate([res.results[c]["out_h"] for c in range(meta["ncores"])], 0)
    return np.ascontiguousarray(out[:n_nodes]).astype(np.float32)


def kernel(**inputs):
    return run(inputs)


# revision 29
# speedup vs baseline: 1.6453x; 1.0751x over previous
"""Trainium2 Bass kernel for nn_GNN_GRU_83519934038653 (GatedGraphConv-style GNN).

Strategy (8 NeuronCores, SPMD, one NEFF):
  - Host: sort edges by dst, shard by dst node-range (1280 nodes/core), build
    int16 gather index tables + per-tile one-hot scatter matrices S (and their
    per-tile transposes ST for the dst-side select), permute weights.
  - h tables in DRAM use a host-chosen row permutation so every SBUF->DRAM h
    write is one contiguous descriptor per partition; the gather index tables
    absorb the permutation.
  - Gathers run as prepare_only descriptor generation on SWDGE queues 1-3
    (hidden under compute) + trigger_dma when the source table is ready;
    pre-staged descriptors drain at ~250 GB/s vs ~40 GB/s gen-paced.
  - A2: one gather of h0[src] shared with step 1; h0[dst] needs no gather at
    all (dst is window-local: one-hot ST matmuls against h0 windows).
  - Phase B x3: wq = We chunk DMA; DVE multiply (h broadcast over o);
    one tree-add level; PE scatter-matmul with one-hot S; per-window tree;
    transposed GRU; AllGather h (except last step).
"""

import os
import sys

for _p in ("/opt/trn_rl_repo", "/root/.axon_site/_ro/trn_rl_repo"):
    if os.path.isdir(_p) and _p not in sys.path:
        sys.path.insert(0, _p)

import numpy as np

import concourse.bass as bass
import concourse.bacc as bacc
import concourse.mybir as mybir
import concourse.tile as tile
import concourse.bass_utils as bass_utils

F32 = mybir.dt.float32
F16 = mybir.dt.float16
F8 = mybir.dt.float8e4
BF16 = mybir.dt.bfloat16
I16 = mybir.dt.int16
AF = mybir.ActivationFunctionType
OP = mybir.AluOpType

N_NODES = 10000
N_EDGES = 160000
D = 32
IN_DIM = 2
STEPS = 3
CORES = 8
NPC = 1280   # padded nodes per core; 8*1280 = 10240
WIN = 128    # scatter window (nodes per PSUM accumulation window)
NCH = 4      # gather chunks (SWDGE queues 0..NCH-1)

S_NP = mybir.dt.np(F8)
import ml_dtypes
_BF16_NP = ml_dtypes.bfloat16

LAST_RESULT = None
LAST_META = None


# --------------------------------------------------------------------------- #
# host-side preparation
# --------------------------------------------------------------------------- #

def _wrap_idx(idx, epc):
    w = idx.reshape(epc // 16, 16).T.astype(np.int16)
    return np.tile(w, (8, 1)).copy()


def _rowmap_A(n):
    """h_full0 row of node n: h0st[p, w] holds node w*128+p, stored p-major."""
    return (n % 128) * 80 + n // 128


def _rowmap_B(n):
    """h_full(step) row of node n after AllGather of p-major cc_in shards."""
    c = n // NPC
    loc = n % NPC
    return c * NPC + (loc % 128) * (NPC // 128) + loc // 128


def host_prep(x, src, dst, W_emb, b_emb, W_theta, b_theta, W_phi, b_phi,
              W1, b1, W2, b2, W_ih, b_ih, W_hh, b_hh,
              n_nodes=N_NODES, npc=NPC, ncores=CORES):
    n_pad = npc * ncores
    wpc = npc // WIN
    nwin = ncores * wpc

    src = np.asarray(src).astype(np.int64)
    dst = np.asarray(dst).astype(np.int64)
    order = np.argsort(dst, kind="stable")
    src_s = src[order]
    dst_s = dst[order]

    win_of_edge = dst_s // WIN
    counts = np.bincount(win_of_edge, minlength=nwin)
    T = max(1, int(np.ceil(counts.max() / 128)))
    ntpc = wpc * T
    epc = ntpc * 128
    assert epc % (16 * NCH) == 0 and ntpc % NCH == 0

    win_start = np.concatenate([[0], np.cumsum(counts)])

    per_core = []
    spread = (np.arange(epc, dtype=np.int64) * 127) % n_pad
    for c in range(ncores):
        src_idx = spread.copy()
        S = np.zeros((128, ntpc, 128), np.float32)
        for w in range(wpc):
            g = c * wpc + w
            e0, e1 = int(win_start[g]), int(win_start[g + 1])
            k = e1 - e0
            if k == 0:
                continue
            base = w * T * 128
            j = np.arange(k)
            src_idx[base + j] = src_s[e0:e1]
            dloc = dst_s[e0:e1] - (c * npc + w * WIN)
            assert (dloc >= 0).all() and (dloc < WIN).all()
            S[j % 128, w * T + j // 128, dloc] = 1.0
        ST = np.ascontiguousarray(S.transpose(2, 1, 0))  # [dloc, tile, e]
        per_core.append({
            "srcA_w": _wrap_idx(_rowmap_A(src_idx), epc),
            "srcB_w": _wrap_idx(_rowmap_B(src_idx), epc),
            "s_mat": S.astype(S_NP),
            "st_mat": ST.astype(S_NP),
            "xT_own": None,
        })

    f32 = np.float32
    f16 = np.float16
    x = np.asarray(x, f32)
    x_pad = np.zeros((n_pad, IN_DIM), f32)
    x_pad[:n_nodes] = x
    xT_aug = np.concatenate([x_pad.T, np.ones((1, n_pad), f32)], 0)  # [3, n_pad]
    for c in range(ncores):
        per_core[c]["xT_own"] = np.ascontiguousarray(
            xT_aug[:, c * npc:(c + 1) * npc])
        per_core[c]["xTo16"] = per_core[c]["xT_own"].astype(f16)

    W_emb_aug = np.concatenate([np.asarray(W_emb, f32),
                                np.asarray(b_emb, f32)[None, :]], 0)  # [3, 32]

    W2p = np.asarray(W2, f32).reshape(64, D, D).transpose(0, 2, 1).reshape(64, D * D)
    b2p = np.asarray(b2, f32).reshape(D, D).T.reshape(D * D)
    W2pa = np.concatenate([W2p, b2p[None, :]], 0).astype(f16)  # [65, 1024]

    shared = {
        "xT_aug": xT_aug,
        "xT16": xT_aug.astype(f16),
        "w_emb": W_emb_aug,
        "w_emb16": W_emb_aug.astype(f16),
        "w_theta": np.asarray(W_theta, f32).astype(f16),
        "w_phi": np.asarray(W_phi, f32).astype(f16),
        "w1": np.asarray(W1, f32).astype(f16),
        "w2pa": W2pa.astype(np.float32).astype(_BF16_NP),
        "w_ih": np.asarray(W_ih, f32),
        "w_hh": np.asarray(W_hh, f32),
        "b_tp": (np.asarray(b_theta, f32) + np.asarray(b_phi, f32))[:, None],
        "b1c": np.asarray(b1, f32)[:, None],
        "b_r": (np.asarray(b_ih, f32)[0:D] + np.asarray(b_hh, f32)[0:D])[:, None],
        "b_z": (np.asarray(b_ih, f32)[D:2 * D] + np.asarray(b_hh, f32)[D:2 * D])[:, None],
        "b_in": np.asarray(b_ih, f32)[2 * D:3 * D][:, None],
        "b_hn": np.asarray(b_hh, f32)[2 * D:3 * D][:, None],
        "idm": np.eye(128, dtype=f32),
        "idm16": np.eye(128).astype(f16),
    }
    meta = dict(T=T, npc=npc, ncores=ncores, n_pad=n_pad, wpc=wpc,
                ntpc=ntpc, epc=epc, steps=STEPS)
    return shared, per_core, meta


# --------------------------------------------------------------------------- #
# device kernel builder
# --------------------------------------------------------------------------- #

def _bcast_mid(ap_base, count):
    aps = [list(p) for p in ap_base.ap]
    new = aps[:-1] + [[0, count]] + [aps[-1]]
    return bass.AP(ap_base.tensor, ap_base.offset, new)


def build_nc(meta):
    T = meta["T"]; npc = meta["npc"]; ncores = meta["ncores"]
    n_pad = meta["n_pad"]; wpc = meta["wpc"]; ntpc = meta["ntpc"]
    epc = meta["epc"]; steps = meta["steps"]
    tpch = ntpc // NCH           # tiles per gather chunk
    epch = tpch * 128            # edges per gather chunk
    assert epch % 16 == 0
    steps_exec = int(os.environ.get("K_STEPS", steps))
    no_a2 = bool(int(os.environ.get("K_NOA2", "0")))
    no_trig = bool(int(os.environ.get("K_NOTRIG", "0")))
    no_sig = bool(int(os.environ.get("K_NOSIG", "0")))
    no_touch = bool(int(os.environ.get("K_NOTOUCH", "0")))

    nc = bacc.Bacc("TRN2", target_bir_lowering=False, debug=False,
                   enable_asserts=False, num_devices=ncores,
                   num_swdge_queues=4)
    global _DBG_NC
    _DBG_NC = nc

    # ---- I/O tensors ----
    t_xT = nc.dram_tensor("xT_aug", [IN_DIM + 1, n_pad], F32, kind="ExternalInput")
    t_xT16 = nc.dram_tensor("xT16", [IN_DIM + 1, n_pad], F16, kind="ExternalInput")
    t_xTo = nc.dram_tensor("xT_own", [IN_DIM + 1, npc], F32, kind="ExternalInput")
    t_xTo16 = nc.dram_tensor("xTo16", [IN_DIM + 1, npc], F16, kind="ExternalInput")
    t_wemb = nc.dram_tensor("w_emb", [IN_DIM + 1, D], F32, kind="ExternalInput")
    t_wemb16 = nc.dram_tensor("w_emb16", [IN_DIM + 1, D], F16, kind="ExternalInput")
    t_wth = nc.dram_tensor("w_theta", [D, D], F16, kind="ExternalInput")
    t_wph = nc.dram_tensor("w_phi", [D, D], F16, kind="ExternalInput")
    t_w1 = nc.dram_tensor("w1", [D, 64], F16, kind="ExternalInput")
    t_w2 = nc.dram_tensor("w2pa", [65, 1024], BF16, kind="ExternalInput")
    t_wih = nc.dram_tensor("w_ih", [D, 3 * D], F32, kind="ExternalInput")
    t_whh = nc.dram_tensor("w_hh", [D, 3 * D], F32, kind="ExternalInput")
    t_btp = nc.dram_tensor("b_tp", [D, 1], F32, kind="ExternalInput")
    t_b1c = nc.dram_tensor("b1c", [64, 1], F32, kind="ExternalInput")
    t_br = nc.dram_tensor("b_r", [D, 1], F32, kind="ExternalInput")
    t_bz = nc.dram_tensor("b_z", [D, 1], F32, kind="ExternalInput")
    t_bin = nc.dram_tensor("b_in", [D, 1], F32, kind="ExternalInput")
    t_bhn = nc.dram_tensor("b_hn", [D, 1], F32, kind="ExternalInput")
    t_idm = nc.dram_tensor("idm", [128, 128], F32, kind="ExternalInput")
    t_idm16 = nc.dram_tensor("idm16", [128, 128], F16, kind="ExternalInput")
    t_srcA = nc.dram_tensor("srcA_w", [128, epc // 16], I16, kind="ExternalInput")
    t_srcB = nc.dram_tensor("srcB_w", [128, epc // 16], I16, kind="ExternalInput")
    t_smat = nc.dram_tensor("s_mat", [128, ntpc, 128], F8, kind="ExternalInput")
    t_stmat = nc.dram_tensor("st_mat", [128, ntpc, 128], F8, kind="ExternalInput")
    t_out = nc.dram_tensor("out_h", [npc, D], F32, kind="ExternalOutput")

    sem_q = [nc.alloc_semaphore(f"gsem_q{q}") for q in range(NCH)]
    psem = nc.alloc_semaphore("gprep_sem")

    with tile.TileContext(nc) as tc:
        with tc.tile_pool(name="dram", bufs=1, space="DRAM") as dpool, \
             tc.tile_pool(name="const", bufs=1) as cpool:
            we_dram = dpool.tile([epc, 1024], F16, name="we_dram")
            h_full0 = dpool.tile([n_pad, 128], F16, name="h_full0")
            h_fulls = [dpool.tile([n_pad, 128], F16, addr_space="Shared",
                                  name=f"h_full{s}") for s in (1, 2)]
            cc_ins = [dpool.tile([npc, 128], F16, name=f"cc_in{s}")
                      for s in range(steps - 1)]

            # resident constants
            idm = cpool.tile([128, 128], F32, name="idm")
            nc.sync.dma_start(idm[:], t_idm.ap())
            idm16 = cpool.tile([128, 128], F16, name="idm16")
            nc.sync.dma_start(idm16[:], t_idm16.ap())
            touch = cpool.tile([1, 2], F16, name="touch")
            tsem = nc.alloc_semaphore("touch_sem")
            S_sb = cpool.tile([128, ntpc * 128], F8, name="S_sb")
            nc.sync.dma_start(S_sb[:], t_smat.ap().rearrange("p t e -> p (t e)"))
            iA = cpool.tile([128, epc // 16], I16, name="iA")
            nc.sync.dma_start(iA[:], t_srcA.ap())
            iB = cpool.tile([128, epc // 16], I16, name="iB")
            nc.sync.dma_start(iB[:], t_srcB.ap())

            def load_const(t, shape, dtype, name):
                s = cpool.tile(shape, dtype, name=name)
                nc.sync.dma_start(s[:], t.ap())
                return s

            xTo_sb = load_const(t_xTo, [IN_DIM + 1, npc], F32, "xTo_sb")
            wemb_sb = load_const(t_wemb, [IN_DIM + 1, D], F32, "wemb_sb")
            wemb16_sb = load_const(t_wemb16, [IN_DIM + 1, D], F16, "wemb16_sb")
            wth_sb = load_const(t_wth, [D, D], F16, "wth_sb")
            wph_sb = load_const(t_wph, [D, D], F16, "wph_sb")
            w1_sb = load_const(t_w1, [D, 64], F16, "w1_sb")
            w2_sb = load_const(t_w2, [65, 1024], BF16, "w2_sb")
            wih_sb = load_const(t_wih, [D, 3 * D], F32, "wih_sb")
            whh_sb = load_const(t_whh, [D, 3 * D], F32, "whh_sb")
            btp_sb = load_const(t_btp, [D, 1], F32, "btp_sb")
            b1c_sb = load_const(t_b1c, [64, 1], F32, "b1c_sb")
            br_sb = load_const(t_br, [D, 1], F32, "br_sb")
            bz_sb = load_const(t_bz, [D, 1], F32, "bz_sb")
            bin_sb = load_const(t_bin, [D, 1], F32, "bin_sb")
            bhn_sb = load_const(t_bhn, [D, 1], F32, "bhn_sb")

            # GRU state (transposed layout), ping-pong across steps
            h_bufs = [cpool.tile([D, npc], F32, name=f"hT{i}") for i in range(2)]
            # gathered h[src] (shared by A2 and each step's DVE stage)
            G = cpool.tile([128, ntpc, 128], F16, name="G")

            def prep_gather(idx_sb, h_src_ap):
                for c in range(NCH):
                    i = nc.gpsimd.dma_gather(
                        G[:, c * tpch:(c + 1) * tpch, :], h_src_ap,
                        idx_sb[:, c * (epch // 16):(c + 1) * (epch // 16)],
                        epch, epch, 128,
                        transpose=False, single_packet=False,
                        prepare_only=True, sem=sem_q[c], queue_num=c)
                    i.then_inc(psem, 1)

            def trigger_gather(round_no, src_ap, prep_args=None):
                # trigger + completion waits in one critical; preps either
                # inline (round 1) or emitted earlier (bare) to hide desc-gen
                with tc.tile_critical():
                    if prep_args is not None:
                        prep_gather(*prep_args)
                    nc.gpsimd.wait_ge(psem, NCH * round_no)
                    for c in range(NCH):
                        nc.gpsimd.trigger_dma(count=1, queue_num=c)
                    for c in range(NCH):
                        nc.gpsimd.wait_ge(sem_q[c], 16 * round_no)

            late_prep = int(os.environ.get("K_LATEPREP", "0"))


            # ---------------- A1 + A2 scoped pool ----------------
            _apool_cm = tc.tile_pool(name="aph", bufs=1)
            apool = _apool_cm.__enter__()
            h0st = apool.tile([128, 80, 128], F16, name="h0st")
            xT16 = apool.tile([IN_DIM + 1, n_pad], F16, name="xT16")
            nc.sync.dma_start(xT16[:], t_xT16.ap())
            # ---------------- A1: h0 ----------------
            with tc.tile_pool(name="pA1", bufs=2, space="PSUM") as pp1:
                nc.vector.memset(h0st[:], 0.0)
                for ch in range(80):
                    ps = pp1.tile([128, D], F32, tag="psh0")
                    nc.tensor.matmul(ps[:], lhsT=xT16[:, ch * 128:(ch + 1) * 128],
                                     rhs=wemb16_sb[:], start=True, stop=True)
                    nc.scalar.copy(h0st[:, ch, 0:D], ps[:])
                # one contiguous-per-partition descriptor per partition
                if not bool(int(os.environ.get("K_NOSTORE", "0"))):
                    nc.sync.dma_start(
                        h_full0[:].rearrange("(p t) f -> p (t f)", p=128), h0st[:])
                # own-window node-major h0 (dst-side stationaries for A2)
                xTo16_sb = apool.tile([IN_DIM + 1, npc], F16, name="xTo16_sb")
                nc.sync.dma_start(xTo16_sb[:], t_xTo16.ap())
                h0own = apool.tile([128, wpc, D], F16, name="h0own")
                for w in range(wpc):
                    ps = pp1.tile([128, D], F32, tag="psh0")
                    nc.tensor.matmul(ps[:], lhsT=xTo16_sb[:, w * 128:(w + 1) * 128],
                                     rhs=wemb16_sb[:], start=True, stop=True)
                    nc.scalar.copy(h0own[:, w, :], ps[:])
                # own-range h0 transposed (fp32 GRU state)
                for c0 in range(0, npc, 512):
                    cn = min(512, npc - c0)
                    ps = pp1.tile([D, 512], F32, tag="pshT")
                    nc.tensor.matmul(ps[:, 0:cn], lhsT=wemb_sb[:],
                                     rhs=xTo_sb[:, c0:c0 + cn], start=True, stop=True)
                    nc.vector.tensor_copy(h_bufs[0][:, c0:c0 + cn], ps[:, 0:cn])

            # gather round 1: h0[src] -> G (waits for h_full0 via prep deps)
            if not no_trig:
                trigger_gather(1, h_full0[0:1, 0:2],
                               prep_args=(iA, h_full0[:, :]))

            # ---------------- A2: edge MLP -> We ----------------
            with tc.tile_pool(name="pHD", bufs=1, space="PSUM") as phd, \
                 tc.tile_pool(name="pHE", bufs=1, space="PSUM") as phe, \
                 tc.tile_pool(name="pW", bufs=2, space="PSUM") as pw, \
                 tc.tile_pool(name="sA2", bufs=3) as sp2, \
                 tc.tile_pool(name="sST", bufs=1) as spst, \
                 tc.tile_pool(name="sWt", bufs=3) as spw:
                ST_sb = spst.tile([128, ntpc * 128], F8, name="ST_sb")
                nc.sync.dma_start(ST_sb[:],
                                  t_stmat.ap().rearrange("p t e -> p (t e)"))
                evac_flip = 0
                for t0 in (range(0, ntpc, 4) if not no_a2 else []):  # 4 tiles = 512 edges
                    tn = 4
                    en = tn * 128
                    # hdT via one-hot ST matmuls (dst is window-local)
                    pshd = phd.tile([D, 512], F32, tag="pshd")
                    j = 0
                    while j < tn:
                        gt = t0 + j
                        w = gt // T
                        j2 = j
                        while j2 < tn and (t0 + j2) // T == w:
                            j2 += 1
                        nc.tensor.matmul(
                            pshd[:, j * 128:j2 * 128],
                            lhsT=h0own[:, w, :],
                            rhs=ST_sb[:, gt * 128:(t0 + j2) * 128],
                            start=True, stop=True)
                        j = j2
                    # hsT for the 4 tiles via one batched PE transpose:
                    # in [128e, (4t, 32d)] -> psum [(4t, 32d), 128e]
                    Gc = sp2.tile([128, 128], F16, tag="Gc")
                    nc.vector.tensor_copy(
                        Gc[:].rearrange("p (t d) -> p t d", t=4),
                        G[:, t0:t0 + 4, 0:D])
                    pshs = phd.tile([128, 128], F16, tag="pshs")
                    nc.tensor.transpose(pshs[:], Gc[:], idm16[:])
                    hsT = sp2.tile([D, 512], F16, tag="hsT")
                    for j in range(tn):
                        nc.vector.tensor_copy(hsT[:, j * 128:(j + 1) * 128],
                                              pshs[j * D:(j + 1) * D, :])
                    dT = sp2.tile([D, 512], F16, tag="dT")
                    nc.vector.tensor_sub(dT[:, 0:en], pshd[:, 0:en], hsT[:, 0:en])
                    # he = relu(Wth dT + Wph hsT + b)
                    psh = phe.tile([D, 512], F32, tag="pshe")
                    nc.tensor.matmul(psh[:, 0:en], lhsT=wth_sb[:],
                                     rhs=dT[:, 0:en], start=True, stop=False)
                    nc.tensor.matmul(psh[:, 0:en], lhsT=wph_sb[:],
                                     rhs=hsT[:, 0:en], start=False, stop=True)
                    he = sp2.tile([D, 512], F16, tag="he")
                    nc.scalar.activation(he[:, 0:en], psh[:, 0:en], AF.Relu,
                                         bias=btp_sb[:])
                    psg = phe.tile([64, 512], F32, tag="psg")
                    nc.tensor.matmul(psg[:, 0:en], lhsT=w1_sb[:], rhs=he[:, 0:en],
                                     start=True, stop=True)
                    ga = sp2.tile([65, 512], BF16, tag="ga")
                    nc.vector.memset(ga[64:65, 0:en], 1.0)
                    nc.scalar.activation(ga[0:64, 0:en], psg[:, 0:en], AF.Relu,
                                         bias=b1c_sb[:])
                    for j in range(tn):
                        gt = t0 + j
                        s0 = j * 128
                        pw0 = pw.tile([128, 512], F32, tag="psw0")
                        pw1 = pw.tile([128, 512], F32, tag="psw1")
                        nc.tensor.matmul(pw0[:], lhsT=ga[:, s0:s0 + 128],
                                         rhs=w2_sb[:, 0:512], start=True, stop=True)
                        nc.tensor.matmul(pw1[:], lhsT=ga[:, s0:s0 + 128],
                                         rhs=w2_sb[:, 512:1024], start=True, stop=True)
                        wt = spw.tile([128, 1024], F16, tag="wt")
                        if evac_flip == 0:
                            nc.scalar.copy(wt[:, 0:512], pw0[:])
                            nc.vector.tensor_copy(wt[:, 512:1024], pw1[:])
                        else:
                            nc.vector.tensor_copy(wt[:, 0:512], pw0[:])
                            nc.scalar.copy(wt[:, 512:1024], pw1[:])
                        evac_flip ^= 1
                        nc.sync.dma_start(
                            we_dram[gt * 128:(gt + 1) * 128, :], wt[:])

            _apool_cm.__exit__(None, None, None)

            # ---------------- Phase B: message passing steps ----------------
            we_view = we_dram[:].rearrange("(t p) f -> p t f", p=128)
            with tc.tile_pool(name="sWq", bufs=4) as swq, \
                 tc.tile_pool(name="sPr", bufs=3) as spr, \
                 tc.tile_pool(name="sP16", bufs=3) as sp16, \
                 tc.tile_pool(name="sWin", bufs=2) as swin, \
                 tc.tile_pool(name="sGru", bufs=1) as sgru, \
                 tc.tile_pool(name="pA", bufs=2, space="PSUM") as ppa, \
                 tc.tile_pool(name="pT", bufs=1, space="PSUM") as ppt, \
                 tc.tile_pool(name="pG", bufs=1, space="PSUM") as ppg:
                for step in range(steps_exec):
                    h_cur = h_bufs[step % 2]
                    h_new = h_bufs[(step + 1) % 2]

                    if step > 0:
                        trigger_gather(step + 1, h_fulls[step - 1][0:1, 0:2],
                                       prep_args=(iB, h_fulls[step - 1][:, :]))

                    aT = sgru.tile([D, npc], F32, tag="aT")
                    psa = None
                    for q0 in range(0, ntpc, 4):
                        k = min(4, ntpc - q0)
                        wq = swq.tile([128, 4, 1024], F16, tag="wq")
                        nc.sync.dma_start(wq[:, 0:k, :], we_view[:, q0:q0 + k, :])
                        prod = spr.tile([128, 4, D, D], F16, tag="prod")
                        base = G[:, q0:q0 + k, 0:D]
                        in1 = _bcast_mid(base, D)
                        nc.vector.tensor_tensor(
                            prod[:, 0:k, :, :],
                            wq[:, 0:k, :].rearrange("p t (o i) -> p t o i", o=D),
                            in1, op=OP.mult)
                        p16 = sp16.tile([128, 4, D, 16], F16, tag="p16")
                        nc.vector.tensor_tensor(
                            p16[:, 0:k, :, :], prod[:, 0:k, :, 0:16],
                            prod[:, 0:k, :, 16:32], op=OP.add)
                        for j in range(k):
                            gt = q0 + j
                            w = gt // T
                            tloc = gt % T
                            if tloc == 0:
                                psa = ppa.tile([128, 512], F32, tag="psa")
                            nc.tensor.matmul(
                                psa[:], lhsT=S_sb[:, gt * 128:(gt + 1) * 128],
                                rhs=p16[:, j, :, :],
                                start=(tloc == 0), stop=(tloc == T - 1))
                            if tloc == T - 1:
                                aw = swin.tile([128, D, 16], F32, tag="aw")
                                nc.scalar.copy(
                                    aw[:], psa[:].rearrange("p (o i) -> p o i", o=D))
                                t8 = swin.tile([128, D, 8], F32, tag="t8")
                                nc.vector.tensor_tensor(t8[:], aw[:, :, 0:8],
                                                        aw[:, :, 8:16], op=OP.add)
                                t4 = swin.tile([128, D, 4], F32, tag="t4")
                                nc.vector.tensor_tensor(t4[:], t8[:, :, 0:4],
                                                        t8[:, :, 4:8], op=OP.add)
                                t2 = swin.tile([128, D, 2], F32, tag="t2")
                                nc.vector.tensor_tensor(t2[:], t4[:, :, 0:2],
                                                        t4[:, :, 2:4], op=OP.add)
                                t1 = swin.tile([128, D], F32, tag="t1")
                                nc.vector.tensor_tensor(t1[:], t2[:, :, 0],
                                                        t2[:, :, 1], op=OP.add)
                                pst = ppt.tile([D, 128], F32, tag="pst")
                                nc.tensor.transpose(pst[:], t1[:], idm[:])
                                nc.vector.tensor_copy(
                                    aT[:, w * 128:(w + 1) * 128], pst[:])

                    # ---- GRU (transposed layout) ----
                    for c0 in range(0, npc, 512):
                        cn = min(512, npc - c0)
                        cs = slice(c0, c0 + cn)
                        pgi = ppg.tile([3 * D, 512], F32, tag="pgi")
                        nc.tensor.matmul(pgi[:, 0:cn], lhsT=wih_sb[:],
                                         rhs=aT[:, cs], start=True, stop=True)
                        pgh = ppg.tile([3 * D, 512], F32, tag="pgh")
                        nc.tensor.matmul(pgh[:, 0:cn], lhsT=whh_sb[:],
                                         rhs=h_cur[:, cs], start=True, stop=True)
                        gh_sb = sgru.tile([3 * D, 512], F32, tag="gh_sb")
                        nc.scalar.copy(gh_sb[:, 0:cn], pgh[:, 0:cn])
                        tr = sgru.tile([D, 512], F32, tag="tr")
                        nc.vector.tensor_add(tr[:, 0:cn], pgi[0:D, 0:cn],
                                             gh_sb[0:D, 0:cn])
                        r = sgru.tile([D, 512], F32, tag="r")
                        nc.scalar.activation(r[:, 0:cn], tr[:, 0:cn], AF.Sigmoid,
                                             bias=br_sb[:])
                        tz = sgru.tile([D, 512], F32, tag="tz")
                        nc.vector.tensor_add(tz[:, 0:cn], pgi[D:2 * D, 0:cn],
                                             gh_sb[D:2 * D, 0:cn])
                        z = sgru.tile([D, 512], F32, tag="z")
                        nc.scalar.activation(z[:, 0:cn], tz[:, 0:cn], AF.Sigmoid,
                                             bias=bz_sb[:])
                        hnb = sgru.tile([D, 512], F32, tag="hnb")
                        nc.vector.tensor_scalar_add(hnb[:, 0:cn],
                                                    gh_sb[2 * D:3 * D, 0:cn],
                                                    bhn_sb[:])
                        rhn = sgru.tile([D, 512], F32, tag="rhn")
                        nc.vector.tensor_mul(rhn[:, 0:cn], r[:, 0:cn], hnb[:, 0:cn])
                        tn_ = sgru.tile([D, 512], F32, tag="tn_")
                        nc.vector.tensor_add(tn_[:, 0:cn], rhn[:, 0:cn],
                                             pgi[2 * D:3 * D, 0:cn])
                        ngate = sgru.tile([D, 512], F32, tag="ngate")
                        nc.scalar.activation(ngate[:, 0:cn], tn_[:, 0:cn], AF.Tanh,
                                             bias=bin_sb[:])
                        hmn = sgru.tile([D, 512], F32, tag="hmn")
                        nc.vector.tensor_sub(hmn[:, 0:cn], h_cur[:, cs],
                                             ngate[:, 0:cn])
                        zh = sgru.tile([D, 512], F32, tag="zh")
                        nc.vector.tensor_mul(zh[:, 0:cn], z[:, 0:cn], hmn[:, 0:cn])
                        nc.vector.tensor_add(h_new[:, cs], ngate[:, 0:cn],
                                             zh[:, 0:cn])

                    # ---- write h out; AllGather (except after last step) ----
                    if step < steps_exec - 1:
                        hst = sgru.tile([128, wpc, 128], F16, tag="hst")
                        if step == 0:
                            nc.vector.memset(hst[:], 0.0)
                        for w in range(wpc):
                            ps2 = ppt.tile([128, D], F32, tag="ps2")
                            nc.tensor.transpose(ps2[:],
                                                h_new[:, w * 128:(w + 1) * 128],
                                                idm[0:D, 0:D])
                            nc.scalar.copy(hst[:, w, 0:D], ps2[:])
                        nc.sync.dma_start(
                            cc_ins[step][:].rearrange("(p w) f -> p (w f)", p=128),
                            hst[:])
                        nc.gpsimd.collective_compute(
                            "AllGather", OP.bypass,
                            replica_groups=[list(range(ncores))],
                            ins=[cc_ins[step][:].opt()],
                            outs=[h_fulls[step][:].opt()])
                    else:
                        ost = sgru.tile([128, wpc, D], F32, tag="ost")
                        for w in range(wpc):
                            ps2 = ppt.tile([128, D], F32, tag="ps2")
                            nc.tensor.transpose(ps2[:],
                                                h_new[:, w * 128:(w + 1) * 128],
                                                idm[0:D, 0:D])
                            nc.scalar.copy(ost[:, w, :], ps2[:])
                        nc.sync.dma_start(
                            t_out.ap().rearrange("(w p) d -> p w d", p=128), ost[:])

        if steps_exec == 0:
            with tc.tile_pool(name="sZ", bufs=1) as sz:
                zst = sz.tile([128, wpc, D], F32, name="zst")
                nc.vector.memset(zst[:], 0.0)
                nc.sync.dma_start(
                    t_out.ap().rearrange("(w p) d -> p w d", p=128), zst[:])
    nc.compile()
    return nc


# --------------------------------------------------------------------------- #
# entry point
# --------------------------------------------------------------------------- #

def run(inputs, n_nodes=N_NODES, npc=NPC, **spmd_kwargs):
    global LAST_RESULT, LAST_META
    shared, per_core, meta = host_prep(**inputs, n_nodes=n_nodes, npc=npc)
    LAST_META = meta
    nc = build_nc(meta)
    in_maps = [dict(shared, **pc) for pc in per_core]
    res = bass_utils.run_bass_kernel_spmd(
        nc, in_maps, core_ids=list(range(meta["ncores"])), **spmd_kwargs)
    LAST_RESULT = res
    out = np.concatenate([res.results[c]["out_h"] for c in range(meta["ncores"])], 0)
    return np.ascontiguousarray(out[:n_nodes]).astype(np.float32)


def kernel(**inputs):
    return run(inputs)
